# revision 1
# baseline (speedup 1.0000x reference)
"""MoE routing mixture kernel for Trainium2 (8 NeuronCores, SPMD data-parallel).

Math: out[b] = sum_k selection_score[b, idx[b,k]] * all_weight[idx[b,k]]
Rewritten as a dense matmul: out = C @ W_flat, where
  C[b,e]    = selection_score[b,e] * |{k : idx[b,k]==e}|      ([2048, 64])
  W_flat    = all_weight.reshape(64, 16384)
Sharding: batch rows split across 8 cores (256 rows each); W replicated.

Raw Bass (no Tile): this toolchain's descriptors carry at most one sync wait
and one sync update each, so all synchronization is standalone wait_ge
instructions plus .then_inc updates, one per instruction.

Pipeline per core:
  SP   : 6 small input DMAs -> 4 W-chunk DMAs -> 16 output DMAs (1 MiB each)
  DVE  : C = score * count(idx==e) per 128-row chunk; C^T copies from PSUM
  PE   : 2 transposes (C -> C^T), then 64 matmuls [64x128]@[64x512] -> PSUM
  ACT  : 64 PSUM->SBUF copies into 16 staging tiles (no slot reuse)
"""

import sys
from contextlib import ExitStack

import numpy as np

sys.path.insert(0, "/opt/trn_rl_repo")

BS, E, TOPK, PL, D = 2048, 64, 8, 32, 512
NF = PL * D  # 16384 flattened prompt*dim
N_CORES = 8
RPC = BS // N_CORES  # 256 rows per core
RCHUNKS = RPC // 128  # 2 row chunks of 128
HALF = NF // 2  # 8192: W stored on-chip as [128, 8192]
WCHUNKS = 8  # W loaded in 8 chunks of [128, 1024]
WCW = HALF // WCHUNKS  # 2048
SLICES = WCW // D  # 4 matmuls (512 cols) per (chunk, half)
NPSUM = 6  # matmul PSUM ring
NGRP = WCHUNKS * RCHUNKS * 2  # 16 staging groups of [128, 2048]

_cache: dict = {}


def _build_program():
    import concourse.bass as bass
    import concourse.mybir as mybir

    f32 = mybir.dt.float32
    nc = bass.Bass()

    scores_d = nc.declare_dram_parameter("scores", [RPC, E], f32, isOutput=False)
    idx_d = nc.declare_dram_parameter("idxf", [RPC, TOPK], f32, isOutput=False)
    # W_flat [64, 16384] host-rearranged to [128, 8192]:
    # partition h*64+e holds cols [h*8192, (h+1)*8192) of expert e.
    wk_d = nc.declare_dram_parameter("wk", [128, HALF], f32, isOutput=False)
    iota_d = nc.declare_dram_parameter("iota", [128, E], f32, isOutput=False)
    ident_d = nc.declare_dram_parameter("ident", [128, 128], f32, isOutput=False)
    out_d = nc.declare_dram_parameter("out", [RPC, NF], f32, isOutput=True)

    ctx = ExitStack()
    with ctx:
        f32r = mybir.dt.float32r
        sb = lambda shape, tag, dt=f32: ctx.enter_context(  # noqa: E731
            nc.sbuf_tensor(tag, shape, dt)
        )
        w_t = sb([128, HALF], "w_t")
        iota_t = sb([128, E], "iota_t")
        ident_t = sb([128, 128], "ident_t")
        sc_t = [sb([128, E], f"sc{r}") for r in range(RCHUNKS)]
        idx_t = [sb([128, TOPK], f"idx{r}") for r in range(RCHUNKS)]
        eqs = [sb([128, E], f"eq{i}") for i in range(TOPK)]
        prs = [sb([128, E], f"pr{i}") for i in range(TOPK // 2)]
        qds = [sb([128, E], f"qd{i}") for i in range(TOPK // 4)]
        cnt = [sb([128, E], f"cnt{r}") for r in range(RCHUNKS)]
        ct = [sb([128, 128], f"ct{r}") for r in range(RCHUNKS)]
        stg = [sb([128, WCW], f"stg{g}") for g in range(NGRP)]

        ctp = [
            ctx.enter_context(nc.psum_tensor(f"ctp{r}", [E, 128], f32))
            for r in range(RCHUNKS)
        ]
        pmm = [
            ctx.enter_context(nc.psum_tensor(f"pmm{i}", [128, D], f32))
            for i in range(NPSUM)
        ]

        s_in = ctx.enter_context(nc.semaphore("s_in"))
        s_w = ctx.enter_context(nc.semaphore("s_w"))
        s_dve = ctx.enter_context(nc.semaphore("s_dve"))
        s_pe = ctx.enter_context(nc.semaphore("s_pe"))
        s_act = ctx.enter_context(nc.semaphore("s_act"))
        s_cpv = ctx.enter_context(nc.semaphore("s_cpv"))
        s_out = ctx.enter_context(nc.semaphore("s_out"))

        # matmul m (PE order) -> (wchunk c, rowchunk rc, half h, slice s)
        def mm_seq():
            m = 0
            for c in range(WCHUNKS):
                for rc in range(RCHUNKS):
                    for h in range(2):
                        for s in range(SLICES):
                            yield m, c, rc, h, s
                            m += 1

        N_MM = WCHUNKS * RCHUNKS * 2 * SLICES  # 64

        block = ctx.enter_context(nc.Block())

        @block.sync
        def _(sp):
            sp.dma_start(out=iota_t[:], in_=iota_d[:]).then_inc(s_in, 16)
            sp.dma_start(out=ident_t[:], in_=ident_d[:]).then_inc(s_in, 16)
            for r in range(RCHUNKS):
                rows = slice(r * 128, (r + 1) * 128)
                sp.dma_start(out=sc_t[r][:], in_=scores_d[rows, :]).then_inc(s_in, 16)
                sp.dma_start(out=idx_t[r][:], in_=idx_d[rows, :]).then_inc(s_in, 16)
            for c in range(WCHUNKS):
                cols = slice(c * WCW, (c + 1) * WCW)
                sp.dma_start(out=w_t[:, cols], in_=wk_d[:, cols]).then_inc(s_w, 16)

        @block.vector
        def _(v):
            v.wait_ge(s_in, 96)
            for r in range(RCHUNKS):
                for k in range(TOPK):
                    v.tensor_scalar(
                        eqs[k][:],
                        iota_t[:],
                        idx_t[r][:, k : k + 1],
                        None,
                        mybir.AluOpType.is_equal,
                    )
                v.drain()
                for i in range(TOPK // 2):
                    v.tensor_add(prs[i][:], eqs[2 * i][:], eqs[2 * i + 1][:])
                v.drain()
                for i in range(TOPK // 4):
                    v.tensor_add(qds[i][:], prs[2 * i][:], prs[2 * i + 1][:])
                v.drain()
                v.tensor_add(cnt[r][:], qds[0][:], qds[1][:])
                v.drain()
                v.tensor_mul(cnt[r][:], cnt[r][:], sc_t[r][:]).then_inc(s_dve, 1)
            for r in range(RCHUNKS):
                v.wait_ge(s_pe, r + 1)
                v.tensor_copy(ct[r][:E, :], ctp[r][:]).then_inc(s_dve, 1)
                v.tensor_copy(ct[r][E:, :], ctp[r][:]).then_inc(s_dve, 1)
            # odd-m PSUM->SBUF copies (evens go to ACT)
            for m, c, rc, h, s in mm_seq():
                if m % 2 == 0:
                    continue
                v.wait_ge(s_pe, RCHUNKS + m + 1)
                gi = c * (RCHUNKS * 2) + rc * 2 + h
                v.tensor_copy(
                    stg[gi][:, s * D : (s + 1) * D], pmm[m % NPSUM][:]
                ).then_inc(s_cpv, 1)

        @block.tensor
        def _(t):
            t.wait_ge(s_in, 96)  # ident
            for r in range(RCHUNKS):
                t.wait_ge(s_dve, r + 1)
                t.transpose(ctp[r][:], cnt[r][:], ident_t[:]).then_inc(s_pe, 1)
            t.wait_ge(s_dve, RCHUNKS + 2 * RCHUNKS)  # all ct copies done
            cur_c = -1
            for m, c, rc, h, s in mm_seq():
                if c != cur_c:
                    t.wait_ge(s_w, 16 * (c + 1))
                    cur_c = c
                if m >= NPSUM:
                    mm = m - NPSUM
                    if mm % 2 == 0:
                        t.wait_ge(s_act, mm // 2 + 1)
                    else:
                        t.wait_ge(s_cpv, mm // 2 + 1)
                pslice = slice(h * E, (h + 1) * E)
                wc = c * WCW + s * D
                t.matmul(
                    pmm[m % NPSUM][:],
                    ct[rc][pslice, :],
                    w_t[pslice, wc : wc + D],
                    start=True,
                    stop=True,
                ).then_inc(s_pe, 1)

        @block.scalar
        def _(a):
            for m, c, rc, h, s in mm_seq():
                if m % 2 == 1:
                    continue
                a.wait_ge(s_pe, RCHUNKS + m + 1)
                gi = c * (RCHUNKS * 2) + rc * 2 + h
                a.copy(
                    stg[gi][:, s * D : (s + 1) * D], pmm[m % NPSUM][:]
                ).then_inc(s_act, 1)

        @block.gpsimd
        def _(gp):
            # Output stores on SWDGE: group gi ready when its 2 ACT + 2 DVE
            # copies are done.
            gi = 0
            for c in range(WCHUNKS):
                for rc in range(RCHUNKS):
                    for h in range(2):
                        rows = slice(rc * 128, (rc + 1) * 128)
                        colbase = h * HALF + c * WCW
                        gp.wait_ge(s_act, (SLICES // 2) * (gi + 1))
                        gp.wait_ge(s_cpv, (SLICES // 2) * (gi + 1))
                        gp.dma_start(
                            out=out_d[rows, colbase : colbase + WCW],
                            in_=stg[gi][:],
                        ).then_inc(s_out, 16)
                        gi += 1
            gp.wait_ge(s_out, 16 * NGRP)

    return nc


def _run(selection_score, expert_indices, all_weight, trace=False):
    from concourse.bass_utils import run_bass_kernel_spmd

    scores = np.ascontiguousarray(np.asarray(selection_score, dtype=np.float32))
    idxf = np.ascontiguousarray(np.asarray(expert_indices).astype(np.float32))
    w = np.asarray(all_weight, dtype=np.float32).reshape(E, NF)
    wk = np.ascontiguousarray(
        w.reshape(E, 2, HALF).transpose(1, 0, 2).reshape(128, HALF)
    )
    iota = np.ascontiguousarray(np.tile(np.arange(E, dtype=np.float32), (128, 1)))
    ident = np.eye(128, dtype=np.float32)

    if "nc" not in _cache:
        _cache["nc"] = _build_program()
    nc = _cache["nc"]

    in_maps = [
        {
            "scores": np.ascontiguousarray(scores[c * RPC : (c + 1) * RPC]),
            "idxf": np.ascontiguousarray(idxf[c * RPC : (c + 1) * RPC]),
            "wk": wk,
            "iota": iota,
            "ident": ident,
        }
        for c in range(N_CORES)
    ]
    r = run_bass_kernel_spmd(nc, in_maps, list(range(N_CORES)), trace=trace)
    full = np.concatenate([r.results[c]["out"] for c in range(N_CORES)], axis=0)
    return full.reshape(BS, PL, D).astype(np.float32, copy=False), r


def kernel(selection_score, expert_indices, all_weight) -> np.ndarray:
    full, _ = _run(selection_score, expert_indices, all_weight, trace=False)
    return full



# revision 5
# speedup vs baseline: 1.5841x; 1.5841x over previous
"""MoE routing mixture kernel for Trainium2 (8 NeuronCores, SPMD).

Math: out[b] = sum_k selection_score[b, idx[b,k]] * all_weight[idx[b,k]]
Rewritten as a dense matmul: out = C @ W_flat, where
  C[b,e]    = selection_score[b,e] * |{k : idx[b,k]==e}|      ([2048, 64])
  W_flat    = all_weight.reshape(64, 16384)

Sharding: 8 cores = 2 row-groups x 4 col-groups. Each core produces a
[1024, 4096] tile of the [2048, 16384] output. The big store is fp16
(DMA-roofline dominated problem: fp32 out would be 16.8 MB/core, fp16
is 8.4 MB; W slice per core is [64, 4096] fp16 = 0.5 MB).

Per-core pipeline (raw Bass, one sync wait / one update per instr):
  SP  : loads (scoresT, idx-packed, iota, ident, 8 W slices) then 8
        row-chunk output stores of [128, 4096] fp16.
  DVE : per 128-row chunk r: 8x tensor_scalar is_equal (fp16, 4x mode)
        into eq[r], add-tree -> cnt[r]; then ct[r-1] = ctp * scoresT
        (the PSUM->SBUF move of C^T fused with the score multiply).
        Also a few PSUM->SBUF fp32->fp16 cast copies.
  PE  : per chunk: transpose cnt[r] -> ctp (fp16 PSUM); then 8 fp16
        matmuls [64,128]^T @ [64,512] -> fp32 PSUM (1 cycle/row).
  ACT : most PSUM->SBUF fp32->fp16 cast copies.
  Pool: remaining cast copies.

Output assembled on host: fp16 tiles -> fp32 [2048, 32, 512].
"""

import sys
from contextlib import ExitStack

import numpy as np

sys.path.insert(0, "/opt/trn_rl_repo")

BS, E, TOPK, PL, D = 2048, 64, 8, 32, 512
NF = PL * D  # 16384 flattened prompt*dim
N_CORES = 8
RG, CG = 2, 4  # row groups x col groups
ROWS = BS // RG  # 1024 rows per core
COLS = NF // CG  # 4096 cols per core
RCH = ROWS // 128  # 8 row chunks
NSL = COLS // D  # 8 matmul slices of 512 cols
NPS = 3  # psum ring of [128, 1024] units (2 slices each)
NU = RCH * (NSL // 2)  # 32 copy units per core

# copy-unit engine assignment: per chunk [ACT, DVE, Pool, ACT]
_UPAT = ("A", "D", "P", "A")

_cache: dict = {}


def _unit_engine(u):
    return _UPAT[u % 4]


def _copy_done(u):
    """(sem_name, value) proving copy of unit u has completed."""
    eng = _unit_engine(u)
    n = sum(1 for v in range(u + 1) if _unit_engine(v) == eng)
    return eng, n


def _build_program():
    import concourse.bass as bass
    import concourse.mybir as mybir

    f16 = mybir.dt.float16
    f32 = mybir.dt.float32
    eq_op = mybir.AluOpType.is_equal
    nc = bass.Bass()

    scoT_d = nc.declare_dram_parameter("scoT", [E, ROWS], f16, isOutput=False)
    # idxp[p, r*8+k] = idx[r*128+p, k] for row chunk r (f32: tensor_scalar
    # is_equal requires an fp32 scalar operand)
    idxp_d = nc.declare_dram_parameter("idxp", [128, RCH * TOPK], f32, isOutput=False)
    iota_d = nc.declare_dram_parameter("iota", [128, E], f16, isOutput=False)
    ident_d = nc.declare_dram_parameter("ident", [128, 128], f16, isOutput=False)
    w_d = nc.declare_dram_parameter("wk", [E, COLS], f16, isOutput=False)
    out_d = nc.declare_dram_parameter("out", [ROWS, COLS], f16, isOutput=True)

    ctx = ExitStack()
    with ctx:
        sb = lambda tag, shape, dt=f16: ctx.enter_context(  # noqa: E731
            nc.sbuf_tensor(tag, shape, dt)
        )
        scoT_t = sb("scoT_t", [E, ROWS])
        idxp_t = sb("idxp_t", [128, RCH * TOPK], f32)
        iota_t = sb("iota_t", [128, E])
        ident_t = sb("ident_t", [128, 128])
        w_t = sb("w_t", [E, COLS])
        eq = [sb(f"eq{r}", [128, TOPK * E]) for r in range(RCH)]
        tr1 = [sb(f"tr1_{i}", [128, 4 * E]) for i in range(2)]
        tr2 = [sb(f"tr2_{i}", [128, 2 * E]) for i in range(2)]
        cnt = [sb(f"cnt{r}", [128, E]) for r in range(RCH)]
        ct = [sb(f"ct{r}", [E, 128]) for r in range(RCH)]
        stg = [sb(f"stg{r}", [128, COLS]) for r in range(RCH)]

        ctp = [
            ctx.enter_context(nc.psum_tensor(f"ctp{i}", [E, 128], f16))
            for i in range(2)
        ]
        pm = [
            ctx.enter_context(nc.psum_tensor(f"pm{i}", [128, 2 * D], f32))
            for i in range(NPS)
        ]

        s_in = ctx.enter_context(nc.semaphore("s_in"))
        s_w = ctx.enter_context(nc.semaphore("s_w"))
        s_cnt = ctx.enter_context(nc.semaphore("s_cnt"))
        s_tp = ctx.enter_context(nc.semaphore("s_tp"))
        s_c = ctx.enter_context(nc.semaphore("s_c"))
        s_mm = ctx.enter_context(nc.semaphore("s_mm"))
        s_cp = {
            "A": ctx.enter_context(nc.semaphore("s_cpa")),
            "D": ctx.enter_context(nc.semaphore("s_cpd")),
            "P": ctx.enter_context(nc.semaphore("s_cpp")),
        }
        s_out = ctx.enter_context(nc.semaphore("s_out"))

        def copy_wait(engobj, u):
            eng, n = _copy_done(u)
            engobj.wait_ge(s_cp[eng], n)

        block = ctx.enter_context(nc.Block())

        @block.sync
        def _(sp):
            sp.dma_start(out=scoT_t[:], in_=scoT_d[:]).then_inc(s_in, 16)
            sp.dma_start(out=idxp_t[:], in_=idxp_d[:]).then_inc(s_in, 16)
            sp.dma_start(out=iota_t[:], in_=iota_d[:]).then_inc(s_in, 16)
            sp.dma_start(out=ident_t[:], in_=ident_d[:]).then_inc(s_in, 16)
            for s in range(NSL):
                cols = slice(s * D, (s + 1) * D)
                sp.dma_start(out=w_t[:, cols], in_=w_d[:, cols]).then_inc(s_w, 16)
            for r in range(RCH):
                sp.wait_ge(s_cp["A"], 2 * (r + 1))
                sp.wait_ge(s_cp["D"], r + 1)
                sp.wait_ge(s_cp["P"], r + 1)
                rows = slice(r * 128, (r + 1) * 128)
                sp.dma_start(out=out_d[rows, :], in_=stg[r][:]).then_inc(s_out, 16)
            sp.wait_ge(s_out, 16 * RCH)

        @block.vector
        def _(v):
            v.wait_ge(s_in, 64)

            def dve_copy(g):
                # PSUM->SBUF cast copy for this core's unit in group g (j==1)
                u = 4 * g + 1
                v.wait_ge(s_mm, 2 * (u + 1))
                v.tensor_copy(
                    stg[g][:, 1 * 1024 : 2 * 1024], pm[u % NPS][:]
                ).then_inc(s_cp["D"], 1)

            for r in range(RCH):
                for k in range(TOPK):
                    c = r * TOPK + k
                    v.tensor_scalar(
                        eq[r][:, k * E : (k + 1) * E],
                        iota_t[:],
                        idxp_t[:, c : c + 1],
                        None,
                        eq_op,
                    )
                v.drain()
                v.tensor_add(tr1[r % 2][:], eq[r][:, : 4 * E], eq[r][:, 4 * E :])
                v.drain()
                v.tensor_add(tr2[r % 2][:], tr1[r % 2][:, : 2 * E], tr1[r % 2][:, 2 * E :])
                v.drain()
                v.tensor_add(cnt[r][:], tr2[r % 2][:, :E], tr2[r % 2][:, E:]).then_inc(
                    s_cnt, 1
                )
                if r >= 1:
                    v.wait_ge(s_tp, r)
                    v.tensor_mul(
                        ct[r - 1][:],
                        ctp[(r - 1) % 2][:],
                        scoT_t[:, (r - 1) * 128 : r * 128],
                    ).then_inc(s_c, 1)
                if r >= 3:
                    dve_copy(r - 3)
            v.wait_ge(s_tp, RCH)
            v.tensor_mul(
                ct[RCH - 1][:],
                ctp[(RCH - 1) % 2][:],
                scoT_t[:, (RCH - 1) * 128 :],
            ).then_inc(s_c, 1)
            for g in range(RCH - 3, RCH):
                dve_copy(g)

        @block.tensor
        def _(t):
            t.wait_ge(s_in, 64)  # ident

            def tp(r):
                if r >= 2:
                    t.wait_ge(s_c, r - 1)  # ctp bank reader (mul of r-2) done
                t.wait_ge(s_cnt, r + 1)
                t.transpose(ctp[r % 2][:], cnt[r][:], ident_t[:]).then_inc(s_tp, 1)

            tp(0)
            tp(1)
            for r in range(RCH):
                t.wait_ge(s_c, r + 1)  # ct[r] ready
                for j in range(NSL // 2):
                    u = r * (NSL // 2) + j
                    if u >= NPS:
                        copy_wait(t, u - NPS)
                    for h in range(2):
                        s = 2 * j + h
                        if r == 0:
                            t.wait_ge(s_w, 16 * (s + 1))
                        t.matmul(
                            pm[u % NPS][:, h * D : (h + 1) * D],
                            ct[r][:],
                            w_t[:, s * D : (s + 1) * D],
                            start=True,
                            stop=True,
                        ).then_inc(s_mm, 1)
                if r + 2 < RCH:
                    tp(r + 2)

        @block.scalar
        def _(a):
            for r in range(RCH):
                for j in (0, 3):
                    u = 4 * r + j
                    a.wait_ge(s_mm, 2 * (u + 1))
                    a.copy(stg[r][:, j * 1024 : (j + 1) * 1024], pm[u % NPS][:]).then_inc(
                        s_cp["A"], 1
                    )

        @block.gpsimd
        def _(gp):
            for r in range(RCH):
                u = 4 * r + 2
                gp.wait_ge(s_mm, 2 * (u + 1))
                gp.tensor_copy(
                    stg[r][:, 2 * 1024 : 3 * 1024], pm[u % NPS][:]
                ).then_inc(s_cp["P"], 1)

    return nc


def _prep_inputs(selection_score, expert_indices, all_weight):
    scores = np.asarray(selection_score, dtype=np.float32)
    idx = np.asarray(expert_indices)
    w = np.asarray(all_weight, dtype=np.float32).reshape(E, NF).astype(np.float16)
    iota = np.ascontiguousarray(np.tile(np.arange(E, dtype=np.float16), (128, 1)))
    ident = np.eye(128, dtype=np.float16)

    in_maps = []
    for core in range(N_CORES):
        rg, cg = divmod(core, CG)
        rsl = slice(rg * ROWS, (rg + 1) * ROWS)
        scoT = np.ascontiguousarray(scores[rsl].T.astype(np.float16))
        idxp = np.ascontiguousarray(
            idx[rsl]
            .astype(np.float32)
            .reshape(RCH, 128, TOPK)
            .transpose(1, 0, 2)
            .reshape(128, RCH * TOPK)
        )
        wk = np.ascontiguousarray(w[:, cg * COLS : (cg + 1) * COLS])
        in_maps.append(
            {"scoT": scoT, "idxp": idxp, "iota": iota, "ident": ident, "wk": wk}
        )
    return in_maps


def _run(selection_score, expert_indices, all_weight, trace=False):
    from concourse.bass_utils import run_bass_kernel_spmd

    in_maps = _prep_inputs(selection_score, expert_indices, all_weight)
    if "nc" not in _cache:
        _cache["nc"] = _build_program()
    nc = _cache["nc"]

    r = run_bass_kernel_spmd(nc, in_maps, list(range(N_CORES)), trace=trace)
    full = np.empty((BS, NF), dtype=np.float32)
    for core in range(N_CORES):
        rg, cg = divmod(core, CG)
        full[rg * ROWS : (rg + 1) * ROWS, cg * COLS : (cg + 1) * COLS] = r.results[
            core
        ]["out"]
    return full.reshape(BS, PL, D), r


def kernel(selection_score, expert_indices, all_weight) -> np.ndarray:
    full, _ = _run(selection_score, expert_indices, all_weight, trace=False)
    return full


# revision 6
# speedup vs baseline: 1.8663x; 1.1781x over previous
"""MoE routing mixture kernel for Trainium2 (8 NeuronCores, SPMD).

Math: out[b] = sum_k selection_score[b, idx[b,k]] * all_weight[idx[b,k]]
Rewritten as a dense matmul: out = C @ W_flat, where
  C[b,e]    = selection_score[b,e] * |{k : idx[b,k]==e}|      ([2048, 64])
  W_flat    = all_weight.reshape(64, 16384)

Sharding: 8 cores = 2 row-groups x 4 col-groups. Each core produces a
[1024, 4096] tile of the [2048, 16384] output. The big store is fp16
(DMA-roofline dominated problem: fp32 out would be 16.8 MB/core, fp16
is 8.4 MB; W slice per core is [64, 4096] fp16 = 0.5 MB).

Per-core pipeline (raw Bass, one sync wait / one update per instr):
  SP  : 2 input loads (idx-packed f32; iota|ident|scoresT packed f16),
        then 8 row-chunk output stores of [128, 4096] fp16.
  ACT : W load on its own HWDGE queue (parallel with SP loads), then
        2 of 4 PSUM->SBUF fp32->fp16 cast copies per chunk.
  DVE : per 128-row chunk r: 8x tensor_scalar is_equal (fp16, 4x mode)
        into eq[r]; ct[r-1] = ctp * scoresT (PSUM->SBUF move of C^T
        fused with the score multiply); add-tree -> cnt[r]; 1 cast
        copy per chunk (lagged 2 groups).
  PE  : per chunk: transpose cnt[r] -> ctp (fp16 PSUM); 8 fp16 matmuls
        [64,128]^T @ [64,512] -> fp32 PSUM (1 cycle/row).
  Pool: 1 cast copy per chunk.

Output assembled on host: fp16 tiles -> fp32 [2048, 32, 512].
"""

import sys
from contextlib import ExitStack

import numpy as np

sys.path.insert(0, "/opt/trn_rl_repo")

BS, E, TOPK, PL, D = 2048, 64, 8, 32, 512
NF = PL * D  # 16384 flattened prompt*dim
N_CORES = 8
RG, CG = 2, 4  # row groups x col groups
ROWS = BS // RG  # 1024 rows per core
COLS = NF // CG  # 4096 cols per core
RCH = ROWS // 128  # 8 row chunks
NSL = COLS // D  # 8 matmul slices of 512 cols
NPS = 3  # psum ring of [128, 1024] units (2 slices each)

# packed small-input layout: [128, 64 iota | 128 ident | 512 scoT2]
PK_IOTA = 0
PK_ID = E
PK_SCO = E + 128
PKW = E + 128 + 512

# copy-unit engine assignment: per chunk [ACT, ACT, Pool, DVE]
_UPAT = ("A", "A", "P", "D")

_cache: dict = {}


def _unit_engine(u):
    return _UPAT[u % 4]


def _copy_done(u):
    """(sem key, value) proving copy of unit u has completed."""
    eng = _unit_engine(u)
    n = sum(1 for v in range(u + 1) if _unit_engine(v) == eng)
    return eng, n


def _build_program():
    import concourse.bass as bass
    import concourse.mybir as mybir

    f16 = mybir.dt.float16
    f32 = mybir.dt.float32
    eq_op = mybir.AluOpType.is_equal
    nc = bass.Bass()

    # idxp[p, r*8+k] = idx[r*128+p, k] for row chunk r (f32: tensor_scalar
    # is_equal requires an fp32 scalar operand)
    idxp_d = nc.declare_dram_parameter("idxp", [128, RCH * TOPK], f32, isOutput=False)
    pk_d = nc.declare_dram_parameter("pk", [128, PKW], f16, isOutput=False)
    w_d = nc.declare_dram_parameter("wk", [E, COLS], f16, isOutput=False)
    out_d = nc.declare_dram_parameter("out", [ROWS, COLS], f16, isOutput=True)

    ctx = ExitStack()
    with ctx:
        sb = lambda tag, shape, dt=f16: ctx.enter_context(  # noqa: E731
            nc.sbuf_tensor(tag, shape, dt)
        )
        idxp_t = sb("idxp_t", [128, RCH * TOPK], f32)
        pk_t = sb("pk_t", [128, PKW])
        w_t = sb("w_t", [E, COLS])
        eq = [sb(f"eq{r}", [128, TOPK * E]) for r in range(RCH)]
        tr1 = [sb(f"tr1_{i}", [128, 4 * E]) for i in range(2)]
        tr2 = [sb(f"tr2_{i}", [128, 2 * E]) for i in range(2)]
        cnt = [sb(f"cnt{r}", [128, E]) for r in range(RCH)]
        ct = [sb(f"ct{r}", [E, 128]) for r in range(RCH)]
        stg = [sb(f"stg{r}", [128, COLS]) for r in range(RCH)]

        iota_ap = pk_t[:, PK_IOTA:PK_ID]
        ident_ap = pk_t[:, PK_ID:PK_SCO]

        def scoT_ap(r):
            # scoT2[p, c]: p<64 -> scores.T[p, c]; p>=64 -> scores.T[p-64, 512+c]
            pbase = (r // 4) * E
            cbase = PK_SCO + (r % 4) * 128
            return pk_t[pbase : pbase + E, cbase : cbase + 128]

        ctp = [
            ctx.enter_context(nc.psum_tensor(f"ctp{i}", [E, 128], f16))
            for i in range(2)
        ]
        pm = [
            ctx.enter_context(nc.psum_tensor(f"pm{i}", [128, 2 * D], f32))
            for i in range(NPS)
        ]

        s_in = ctx.enter_context(nc.semaphore("s_in"))
        s_w = ctx.enter_context(nc.semaphore("s_w"))
        s_cnt = ctx.enter_context(nc.semaphore("s_cnt"))
        s_tp = ctx.enter_context(nc.semaphore("s_tp"))
        s_c = ctx.enter_context(nc.semaphore("s_c"))
        s_mm = ctx.enter_context(nc.semaphore("s_mm"))
        s_cp = {
            "A": ctx.enter_context(nc.semaphore("s_cpa")),
            "D": ctx.enter_context(nc.semaphore("s_cpd")),
            "P": ctx.enter_context(nc.semaphore("s_cpp")),
        }
        s_out = ctx.enter_context(nc.semaphore("s_out"))

        def copy_wait(engobj, u):
            eng, n = _copy_done(u)
            engobj.wait_ge(s_cp[eng], n)

        block = ctx.enter_context(nc.Block())

        @block.sync
        def _(sp):
            sp.dma_start(out=idxp_t[:], in_=idxp_d[:]).then_inc(s_in, 16)
            sp.dma_start(out=pk_t[:], in_=pk_d[:]).then_inc(s_in, 16)
            for r in range(RCH):
                sp.wait_ge(s_cp["A"], 2 * (r + 1))
                sp.wait_ge(s_cp["P"], r + 1)
                sp.wait_ge(s_cp["D"], r + 1)
                rows = slice(r * 128, (r + 1) * 128)
                sp.dma_start(out=out_d[rows, :], in_=stg[r][:]).then_inc(s_out, 16)
            sp.wait_ge(s_out, 16 * RCH)

        @block.vector
        def _(v):
            v.wait_ge(s_in, 16)  # idxp (first load)

            def dve_copy(g):
                # PSUM->SBUF cast copy for this core's unit in group g (j==3)
                u = 4 * g + 3
                v.wait_ge(s_mm, 2 * (u + 1))
                v.tensor_copy(
                    stg[g][:, 3 * 1024 : 4 * 1024], pm[u % NPS][:]
                ).then_inc(s_cp["D"], 1)

            for r in range(RCH):
                if r == 0:
                    # iota arrives with pk (second load)
                    v.wait_ge(s_in, 32)
                for k in range(TOPK):
                    c = r * TOPK + k
                    v.tensor_scalar(
                        eq[r][:, k * E : (k + 1) * E],
                        iota_ap,
                        idxp_t[:, c : c + 1],
                        None,
                        eq_op,
                    )
                if r >= 1:
                    v.wait_ge(s_tp, r)
                    v.tensor_mul(
                        ct[r - 1][:], ctp[(r - 1) % 2][:], scoT_ap(r - 1)
                    ).then_inc(s_c, 1)
                v.drain()
                v.tensor_add(tr1[r % 2][:], eq[r][:, : 4 * E], eq[r][:, 4 * E :])
                v.drain()
                v.tensor_add(
                    tr2[r % 2][:], tr1[r % 2][:, : 2 * E], tr1[r % 2][:, 2 * E :]
                )
                v.drain()
                v.tensor_add(cnt[r][:], tr2[r % 2][:, :E], tr2[r % 2][:, E:]).then_inc(
                    s_cnt, 1
                )
                if r >= 2:
                    dve_copy(r - 2)
            v.wait_ge(s_tp, RCH)
            v.tensor_mul(
                ct[RCH - 1][:], ctp[(RCH - 1) % 2][:], scoT_ap(RCH - 1)
            ).then_inc(s_c, 1)
            for g in range(RCH - 2, RCH):
                dve_copy(g)

        @block.tensor
        def _(t):
            t.wait_ge(s_in, 32)  # ident in pk

            def tp(r):
                if r >= 2:
                    t.wait_ge(s_c, r - 1)  # ctp bank reader (mul of r-2) done
                t.wait_ge(s_cnt, r + 1)
                t.transpose(ctp[r % 2][:], cnt[r][:], ident_ap).then_inc(s_tp, 1)

            tp(0)
            tp(1)
            for r in range(RCH):
                t.wait_ge(s_c, r + 1)  # ct[r] ready
                if r == 0:
                    t.wait_ge(s_w, 16)
                for j in range(NSL // 2):
                    u = r * (NSL // 2) + j
                    if u >= NPS:
                        copy_wait(t, u - NPS)
                    for h in range(2):
                        s = 2 * j + h
                        t.matmul(
                            pm[u % NPS][:, h * D : (h + 1) * D],
                            ct[r][:],
                            w_t[:, s * D : (s + 1) * D],
                            start=True,
                            stop=True,
                        ).then_inc(s_mm, 1)
                if r + 2 < RCH:
                    tp(r + 2)

        @block.scalar
        def _(a):
            a.dma_start(out=w_t[:], in_=w_d[:]).then_inc(s_w, 16)
            for r in range(RCH):
                for j in (0, 1):
                    u = 4 * r + j
                    a.wait_ge(s_mm, 2 * (u + 1))
                    a.copy(
                        stg[r][:, j * 1024 : (j + 1) * 1024], pm[u % NPS][:]
                    ).then_inc(s_cp["A"], 1)

        @block.gpsimd
        def _(gp):
            for r in range(RCH):
                u = 4 * r + 2
                gp.wait_ge(s_mm, 2 * (u + 1))
                gp.tensor_copy(
                    stg[r][:, 2 * 1024 : 3 * 1024], pm[u % NPS][:]
                ).then_inc(s_cp["P"], 1)

    return nc


def _prep_inputs(selection_score, expert_indices, all_weight):
    scores = np.asarray(selection_score, dtype=np.float32)
    idx = np.asarray(expert_indices)
    w = np.asarray(all_weight, dtype=np.float32).reshape(E, NF).astype(np.float16)
    iota = np.tile(np.arange(E, dtype=np.float16), (128, 1))
    ident = np.eye(128, dtype=np.float16)

    in_maps = []
    for core in range(N_CORES):
        rg, cg = divmod(core, CG)
        rsl = slice(rg * ROWS, (rg + 1) * ROWS)
        scoT = scores[rsl].T.astype(np.float16)  # [64, 1024]
        pk = np.zeros((128, PKW), dtype=np.float16)
        pk[:, PK_IOTA:PK_ID] = iota
        pk[:, PK_ID:PK_SCO] = ident
        pk[:E, PK_SCO:] = scoT[:, :512]
        pk[E:, PK_SCO:] = scoT[:, 512:]
        idxp = np.ascontiguousarray(
            idx[rsl]
            .astype(np.float32)
            .reshape(RCH, 128, TOPK)
            .transpose(1, 0, 2)
            .reshape(128, RCH * TOPK)
        )
        wk = np.ascontiguousarray(w[:, cg * COLS : (cg + 1) * COLS])
        in_maps.append({"idxp": idxp, "pk": pk, "wk": wk})
    return in_maps


def _run(selection_score, expert_indices, all_weight, trace=False):
    from concourse.bass_utils import run_bass_kernel_spmd

    in_maps = _prep_inputs(selection_score, expert_indices, all_weight)
    if "nc" not in _cache:
        _cache["nc"] = _build_program()
    nc = _cache["nc"]

    r = run_bass_kernel_spmd(nc, in_maps, list(range(N_CORES)), trace=trace)
    full = np.empty((BS, NF), dtype=np.float32)
    for core in range(N_CORES):
        rg, cg = divmod(core, CG)
        full[rg * ROWS : (rg + 1) * ROWS, cg * COLS : (cg + 1) * COLS] = r.results[
            core
        ]["out"]
    return full.reshape(BS, PL, D), r


def kernel(selection_score, expert_indices, all_weight) -> np.ndarray:
    full, _ = _run(selection_score, expert_indices, all_weight, trace=False)
    return full


# revision 8
# speedup vs baseline: 1.9387x; 1.0388x over previous
"""MoE routing mixture kernel for Trainium2 (8 NeuronCores, SPMD).

Math: out[b] = sum_k selection_score[b, idx[b,k]] * all_weight[idx[b,k]]
Rewritten as a dense matmul: out = C @ W_flat, where
  C[b,e]    = selection_score[b,e] * |{k : idx[b,k]==e}|      ([2048, 64])
  W_flat    = all_weight.reshape(64, 16384)

Sharding: 8 cores = 2 row-groups x 4 col-groups. Each core produces a
[1024, 4096] tile of the [2048, 16384] output. The big store is fp16
(the problem is DMA-roofline bound: fp32 out would be 16.8 MB/core,
fp16 is 8.4 MB; W slice per core is [64, 4096] fp16 = 0.5 MB).

Per-core pipeline (raw Bass):
  Pool: issues the single packed small-input load via SWDGE at t=0
        (iota | ident | scoresT | idx-bits packed into one fp16 tensor;
        idx scalars are read through a f32 bitcast view), then 2 of the
        4 PSUM->SBUF fp32->fp16 cast copies per row chunk.
  SP  : W load, then 32 per-unit output stores of [128, 1024] fp16.
  DVE : per 128-row chunk r: 8x tensor_scalar is_equal (fp16, 4x DVE
        mode), one strided tensor_reduce summing the 8 one-hot maps,
        then ct[r] = ctp * scoresT (the PSUM->SBUF move of C^T fused
        with the score multiply).
  PE  : per chunk: transpose cnt[r] -> ctp (fp16 PSUM); 8 fp16 matmuls
        [64,128]^T @ [64,512] -> fp32 PSUM (1 cycle/row).
  ACT : 2.5 of the 4 cast copies per chunk.

Output assembled on host: fp16 tiles -> fp32 [2048, 32, 512].
"""

import sys
from contextlib import ExitStack

import numpy as np

sys.path.insert(0, "/opt/trn_rl_repo")

BS, E, TOPK, PL, D = 2048, 64, 8, 32, 512
NF = PL * D  # 16384 flattened prompt*dim
N_CORES = 8
RG, CG = 2, 4  # row groups x col groups
ROWS = BS // RG  # 1024 rows per core
COLS = NF // CG  # 4096 cols per core
RCH = ROWS // 128  # 8 row chunks
NSL = COLS // D  # 8 matmul slices of 512 cols
NPS = 3  # psum ring of [128, 1024] units (2 slices each)

# packed small-input layout (fp16 cols):
# [64 iota | 128 ident | 512 scoT2 | 128 idx-as-f32-bits]
PK_IOTA = 0
PK_ID = E
PK_SCO = E + 128
PK_IDX = E + 128 + 512
PKW = PK_IDX + 2 * RCH * TOPK

_cache: dict = {}


def _build_program():
    import concourse.bass as bass
    import concourse.mybir as mybir

    f16 = mybir.dt.float16
    f32 = mybir.dt.float32
    eq_op = mybir.AluOpType.is_equal
    nc = bass.Bass()

    pk_d = nc.declare_dram_parameter("pk", [128, PKW], f16, isOutput=False)
    w_d = nc.declare_dram_parameter("wk", [E, COLS], f16, isOutput=False)
    out_d = nc.declare_dram_parameter("out", [ROWS, COLS], f16, isOutput=True)

    ctx = ExitStack()
    with ctx:
        sb = lambda tag, shape, dt=f16: ctx.enter_context(  # noqa: E731
            nc.sbuf_tensor(tag, shape, dt)
        )
        pk_t = sb("pk_t", [128, PKW])
        w_t = sb("w_t", [E, COLS])
        eq = [sb(f"eq{r}", [128, TOPK * E]) for r in range(RCH)]
        cnt = [sb(f"cnt{r}", [128, E]) for r in range(RCH)]
        ct = [sb(f"ct{r}", [E, 128]) for r in range(RCH)]
        stg = [sb(f"stg{r}", [128, COLS]) for r in range(RCH)]

        iota_ap = pk_t[:, PK_IOTA:PK_ID]
        ident_ap = pk_t[:, PK_ID:PK_SCO]

        def idx_scalar(r, k):
            c = PK_IDX + 2 * (r * TOPK + k)
            return pk_t[:, c : c + 2].bitcast(f32)

        def scoT_ap(r):
            # scoT2[p, c]: p<64 -> scores.T[p, c]; p>=64 -> scores.T[p-64, 512+c]
            pbase = (r // 4) * E
            cbase = PK_SCO + (r % 4) * 128
            return pk_t[pbase : pbase + E, cbase : cbase + 128]

        ctp = [
            ctx.enter_context(nc.psum_tensor(f"ctp{i}", [E, 128], f16))
            for i in range(2)
        ]
        pm = [
            ctx.enter_context(nc.psum_tensor(f"pm{i}", [128, 2 * D], f32))
            for i in range(NPS)
        ]

        s_in = ctx.enter_context(nc.semaphore("s_in"))
        s_w = ctx.enter_context(nc.semaphore("s_w"))
        s_cnt = ctx.enter_context(nc.semaphore("s_cnt"))
        s_tp = ctx.enter_context(nc.semaphore("s_tp"))
        s_c = ctx.enter_context(nc.semaphore("s_c"))
        s_mm = ctx.enter_context(nc.semaphore("s_mm"))
        s_cpa = ctx.enter_context(nc.semaphore("s_cpa"))
        s_cpp = ctx.enter_context(nc.semaphore("s_cpp"))
        s_out = ctx.enter_context(nc.semaphore("s_out"))

        # copy schedule per group r (all reactive, wait on s_mm):
        #   ACT : u0 full [0:1024]   after pair 0   -> s_cpa = 3r+1
        #         u1 full [1024:2048] after pair 1  -> s_cpa = 3r+2
        #         u3 half [3072:3584] after pair 3  -> s_cpa = 3r+3
        #   Pool: u2 full [2048:3072] after pair 2  -> s_cpp = 2r+1
        #         u3 half [3584:4096] after pair 3  -> s_cpp = 2r+2
        def unit_done_waits(engobj, u):
            r, j = divmod(u, 4)
            if j == 0:
                engobj.wait_ge(s_cpa, 3 * r + 1)
            elif j == 1:
                engobj.wait_ge(s_cpa, 3 * r + 2)
            elif j == 2:
                engobj.wait_ge(s_cpp, 2 * r + 1)
            else:
                engobj.wait_ge(s_cpa, 3 * r + 3)
                engobj.wait_ge(s_cpp, 2 * r + 2)

        block = ctx.enter_context(nc.Block())

        @block.sync
        def _(sp):
            sp.dma_start(out=w_t[:], in_=w_d[:]).then_inc(s_w, 16)
            for u in range(4 * RCH):
                r, j = divmod(u, 4)
                unit_done_waits(sp, u)
                rows = slice(r * 128, (r + 1) * 128)
                cols = slice(j * 1024, (j + 1) * 1024)
                sp.dma_start(out=out_d[rows, cols], in_=stg[r][:, cols]).then_inc(
                    s_out, 16
                )
            sp.wait_ge(s_out, 16 * 4 * RCH)

        @block.vector
        def _(v):
            v.wait_ge(s_in, 16)
            for r in range(RCH):
                for k in range(TOPK):
                    v.tensor_scalar(
                        eq[r][:, k * E : (k + 1) * E],
                        iota_ap,
                        idx_scalar(r, k),
                        None,
                        eq_op,
                    )
                v.drain()
                # counts are small integers (<= 8): exact in fp16
                with nc.allow_low_precision(reason="counts <= 8 are exact in fp16"):
                    v.tensor_reduce(
                        cnt[r][:],
                        eq[r][:].rearrange("p (k e) -> p e k", k=TOPK),
                        mybir.AxisListType.X,
                        mybir.AluOpType.add,
                    ).then_inc(s_cnt, 1)
                v.wait_ge(s_tp, r + 1)
                v.tensor_mul(ct[r][:], ctp[r % 2][:], scoT_ap(r)).then_inc(s_c, 1)

        @block.tensor
        def _(t):
            t.wait_ge(s_in, 16)  # ident

            def tp(r):
                if r >= 2:
                    t.wait_ge(s_c, r - 1)  # ctp bank reader (mul of r-2) done
                t.wait_ge(s_cnt, r + 1)
                t.transpose(ctp[r % 2][:], cnt[r][:], ident_ap).then_inc(s_tp, 1)

            for r in range(RCH):
                tp(r)
                t.wait_ge(s_c, r + 1)  # ct[r] ready
                if r == 0:
                    t.wait_ge(s_w, 16)
                for j in range(NSL // 2):
                    u = r * (NSL // 2) + j
                    if u >= NPS:
                        unit_done_waits(t, u - NPS)
                    for h in range(2):
                        s = 2 * j + h
                        t.matmul(
                            pm[u % NPS][:, h * D : (h + 1) * D],
                            ct[r][:],
                            w_t[:, s * D : (s + 1) * D],
                            start=True,
                            stop=True,
                        ).then_inc(s_mm, 1)

        @block.scalar
        def _(a):
            for r in range(RCH):
                base = 8 * r
                a.wait_ge(s_mm, base + 2)
                a.copy(stg[r][:, 0:1024], pm[(4 * r) % NPS][:]).then_inc(s_cpa, 1)
                a.wait_ge(s_mm, base + 4)
                a.copy(stg[r][:, 1024:2048], pm[(4 * r + 1) % NPS][:]).then_inc(
                    s_cpa, 1
                )
                a.wait_ge(s_mm, base + 8)
                a.copy(
                    stg[r][:, 3072:3584], pm[(4 * r + 3) % NPS][:, 0:512]
                ).then_inc(s_cpa, 1)

        @block.gpsimd
        def _(gp):
            gp.dma_start(out=pk_t[:], in_=pk_d[:]).then_inc(s_in, 16)
            for r in range(RCH):
                base = 8 * r
                gp.wait_ge(s_mm, base + 6)
                gp.tensor_copy(stg[r][:, 2048:3072], pm[(4 * r + 2) % NPS][:]).then_inc(
                    s_cpp, 1
                )
                gp.wait_ge(s_mm, base + 8)
                gp.tensor_copy(
                    stg[r][:, 3584:4096], pm[(4 * r + 3) % NPS][:, 512:1024]
                ).then_inc(s_cpp, 1)

    return nc


def _prep_inputs(selection_score, expert_indices, all_weight):
    scores = np.asarray(selection_score, dtype=np.float32)
    idx = np.asarray(expert_indices)
    w = np.asarray(all_weight, dtype=np.float32).reshape(E, NF).astype(np.float16)
    iota = np.tile(np.arange(E, dtype=np.float16), (128, 1))
    ident = np.eye(128, dtype=np.float16)

    in_maps = []
    for core in range(N_CORES):
        rg, cg = divmod(core, CG)
        rsl = slice(rg * ROWS, (rg + 1) * ROWS)
        scoT = scores[rsl].T.astype(np.float16)  # [64, 1024]
        idxp = np.ascontiguousarray(
            idx[rsl]
            .astype(np.float32)
            .reshape(RCH, 128, TOPK)
            .transpose(1, 0, 2)
            .reshape(128, RCH * TOPK)
        )
        pk = np.zeros((128, PKW), dtype=np.float16)
        pk[:, PK_IOTA:PK_ID] = iota
        pk[:, PK_ID:PK_SCO] = ident
        pk[:E, PK_SCO:PK_IDX] = scoT[:, :512]
        pk[E:, PK_SCO:PK_IDX] = scoT[:, 512:]
        pk[:, PK_IDX:] = idxp.view(np.float16)
        wk = np.ascontiguousarray(w[:, cg * COLS : (cg + 1) * COLS])
        in_maps.append({"pk": pk, "wk": wk})
    return in_maps


def _run(selection_score, expert_indices, all_weight, trace=False):
    from concourse.bass_utils import run_bass_kernel_spmd

    in_maps = _prep_inputs(selection_score, expert_indices, all_weight)
    if "nc" not in _cache:
        _cache["nc"] = _build_program()
    nc = _cache["nc"]

    r = run_bass_kernel_spmd(nc, in_maps, list(range(N_CORES)), trace=trace)
    full = np.empty((BS, NF), dtype=np.float32)
    for core in range(N_CORES):
        rg, cg = divmod(core, CG)
        full[rg * ROWS : (rg + 1) * ROWS, cg * COLS : (cg + 1) * COLS] = r.results[
            core
        ]["out"]
    return full.reshape(BS, PL, D), r


def kernel(selection_score, expert_indices, all_weight) -> np.ndarray:
    full, _ = _run(selection_score, expert_indices, all_weight, trace=False)
    return full


# revision 17
# speedup vs baseline: 2.0233x; 1.0437x over previous
"""MoE routing mixture kernel for Trainium2 (8 NeuronCores, SPMD).

Math: out[b] = sum_k selection_score[b, idx[b,k]] * all_weight[idx[b,k]]
Rewritten as a dense matmul: out = C @ W_flat, where
  C[b,e]    = selection_score[b,e] * |{k : idx[b,k]==e}|      ([2048, 64])
  W_flat    = all_weight.reshape(64, 16384)

Sharding: 8 cores = 2 row-groups x 4 col-groups. Each core produces a
[1024, 4096] tile of the [2048, 16384] output. The big store is fp16
(the problem is DMA-roofline bound: fp32 out would be 16.8 MB/core,
fp16 is 8.4 MB; W slice per core is [64, 4096] fp16 = 0.5 MB).

Per-core pipeline (raw Bass):
  Pool: issues the single packed small-input load via SWDGE at t=0
        (iota | ident | scoresT | idx-bits packed into one fp16 tensor;
        idx scalars are read through a f32 bitcast view), then 2 of the
        4 PSUM->SBUF fp32->fp16 cast copies per row chunk.
  SP  : W load, then 32 per-unit output stores of [128, 1024] fp16.
  DVE : per 128-row chunk r: 8x tensor_scalar is_equal (fp16, 4x DVE
        mode), one strided tensor_reduce summing the 8 one-hot maps,
        then ct[r] = ctp * scoresT (the PSUM->SBUF move of C^T fused
        with the score multiply).
  PE  : per chunk: transpose cnt[r] -> ctp (fp16 PSUM); 8 fp16 matmuls
        [64,128]^T @ [64,512] -> fp32 PSUM (1 cycle/row).
  ACT : 2.5 of the 4 cast copies per chunk.

Output assembled on host: fp16 tiles -> fp32 [2048, 32, 512].
"""

import sys
from contextlib import ExitStack

import numpy as np

sys.path.insert(0, "/opt/trn_rl_repo")

BS, E, TOPK, PL, D = 2048, 64, 8, 32, 512
NF = PL * D  # 16384 flattened prompt*dim
N_CORES = 8
RG, CG = 2, 4  # row groups x col groups
ROWS = BS // RG  # 1024 rows per core
COLS = NF // CG  # 4096 cols per core
RCH = ROWS // 128  # 8 row chunks
NSL = COLS // D  # 8 matmul slices of 512 cols
NPS = 3  # psum ring of [128, 1024] units (2 slices each)

# packed small-input layout (fp16 cols), loaded as two DMAs:
#   mini [0 : 192)   = [64 iota | 128 idx-as-f32-bits]   (unblocks DVE eqs)
#   rest [192 : 832) = [128 ident | 512 scoT2]
PK_IOTA = 0
PK_IDX = E
PK_ID = E + 2 * RCH * TOPK
PK_SCO = PK_ID + 128
PKW = PK_SCO + 512
PK_MINI = PK_ID  # boundary between the two loads

_cache: dict = {}


def _build_program():
    import concourse.bass as bass
    import concourse.mybir as mybir

    f16 = mybir.dt.float16
    f32 = mybir.dt.float32
    eq_op = mybir.AluOpType.is_equal
    nc = bass.Bass()

    pk_d = nc.declare_dram_parameter("pk", [128, PKW], f16, isOutput=False)
    w_d = nc.declare_dram_parameter("wk", [E, COLS], f16, isOutput=False)
    out_d = nc.declare_dram_parameter("out", [ROWS, COLS], f16, isOutput=True)

    ctx = ExitStack()
    with ctx:
        sb = lambda tag, shape, dt=f16: ctx.enter_context(  # noqa: E731
            nc.sbuf_tensor(tag, shape, dt)
        )
        pk_t = sb("pk_t", [128, PKW])
        w_t = sb("w_t", [E, COLS])
        eq = [sb(f"eq{r}", [128, TOPK * E]) for r in range(RCH)]
        cnt = [sb(f"cnt{r}", [128, E]) for r in range(RCH)]
        ct = [sb(f"ct{r}", [E, 128]) for r in range(RCH)]
        stg = [sb(f"stg{r}", [128, COLS]) for r in range(RCH)]

        iota_ap = pk_t[:, PK_IOTA:PK_IDX]
        ident_ap = pk_t[:, PK_ID:PK_SCO]

        def idx_scalar(r, k):
            c = PK_IDX + 2 * (r * TOPK + k)
            return pk_t[:, c : c + 2].bitcast(f32)

        def scoT_ap(r):
            # scoT2[p, c]: p<64 -> scores.T[p, c]; p>=64 -> scores.T[p-64, 512+c]
            pbase = (r // 4) * E
            cbase = PK_SCO + (r % 4) * 128
            return pk_t[pbase : pbase + E, cbase : cbase + 128]

        ctp = [
            ctx.enter_context(nc.psum_tensor(f"ctp{i}", [E, 128], f16))
            for i in range(2)
        ]
        pm = [
            ctx.enter_context(nc.psum_tensor(f"pm{i}", [128, 2 * D], f32))
            for i in range(NPS)
        ]

        s_in = ctx.enter_context(nc.semaphore("s_in"))
        s_w = ctx.enter_context(nc.semaphore("s_w"))
        s_cnt = ctx.enter_context(nc.semaphore("s_cnt"))
        s_tp = ctx.enter_context(nc.semaphore("s_tp"))
        s_c = ctx.enter_context(nc.semaphore("s_c"))
        s_mm = ctx.enter_context(nc.semaphore("s_mm"))
        s_cpa = ctx.enter_context(nc.semaphore("s_cpa"))
        s_cpp = ctx.enter_context(nc.semaphore("s_cpp"))
        s_out = ctx.enter_context(nc.semaphore("s_out"))

        s_cpd = ctx.enter_context(nc.semaphore("s_cpd"))

        # copy schedule per group r (all reactive, wait on s_mm):
        #   ACT : u0 full [0:1024]   after pair 0   -> s_cpa = 3r+1
        #         (r=0: u0 is split ACT [0:512] + DVE [512:1024])
        #         u1 full [1024:2048] after pair 1  -> s_cpa = 3r+2
        #         u3 half [3072:3584] after pair 3  -> s_cpa = 3r+3
        #   Pool: u2 full [2048:3072] after pair 2  -> s_cpp = 2r+1
        #         u3 half [3584:4096] after pair 3  -> s_cpp = 2r+2
        def unit_done_waits(engobj, u):
            r, j = divmod(u, 4)
            if j == 0:
                engobj.wait_ge(s_cpa, 3 * r + 1)
                if r == 0:
                    engobj.wait_ge(s_cpd, 1)
            elif j == 1:
                engobj.wait_ge(s_cpa, 3 * r + 2)
            elif j == 2:
                engobj.wait_ge(s_cpp, 2 * r + 1)
            else:
                engobj.wait_ge(s_cpa, 3 * r + 3)
                engobj.wait_ge(s_cpp, 2 * r + 2)

        block = ctx.enter_context(nc.Block())

        @block.sync
        def _(sp):
            sp.dma_start(out=pk_t[:, :PK_MINI], in_=pk_d[:, :PK_MINI]).then_inc(
                s_in, 16
            )
            sp.dma_start(out=pk_t[:, PK_MINI:], in_=pk_d[:, PK_MINI:]).then_inc(
                s_in, 16
            )
            sp.dma_start(out=w_t[:], in_=w_d[:]).then_inc(s_w, 16)
            for u in range(4 * RCH):
                r, j = divmod(u, 4)
                unit_done_waits(sp, u)
                rows = slice(r * 128, (r + 1) * 128)
                cols = slice(j * 1024, (j + 1) * 1024)
                sp.dma_start(out=out_d[rows, cols], in_=stg[r][:, cols]).then_inc(
                    s_out, 16
                )
            sp.wait_ge(s_out, 16 * 4 * RCH)

        @block.vector
        def _(v):
            v.wait_ge(s_in, 16)  # mini load: iota + idx bits
            for r in range(RCH):
                for k in range(TOPK):
                    v.tensor_scalar(
                        eq[r][:, k * E : (k + 1) * E],
                        iota_ap,
                        idx_scalar(r, k),
                        None,
                        eq_op,
                    )
                if r == 1:
                    # fill accelerator: second half of chunk 0's first copy
                    # unit (ACT does [0:512]); DVE is otherwise idle here
                    v.wait_ge(s_mm, 2)
                    v.tensor_copy(stg[0][:, 512:1024], pm[0][:, 512:1024]).then_inc(
                        s_cpd, 1
                    )
                v.drain()
                # counts are small integers (<= 8): exact in fp16
                with nc.allow_low_precision(reason="counts <= 8 are exact in fp16"):
                    v.tensor_reduce(
                        cnt[r][:],
                        eq[r][:].rearrange("p (k e) -> p e k", k=TOPK),
                        mybir.AxisListType.X,
                        mybir.AluOpType.add,
                    ).then_inc(s_cnt, 1)
                if r == 0:
                    v.wait_ge(s_in, 32)  # scoT arrives with the second load
                v.wait_ge(s_tp, r + 1)
                v.tensor_mul(ct[r][:], ctp[r % 2][:], scoT_ap(r)).then_inc(s_c, 1)

        @block.tensor
        def _(t):
            t.wait_ge(s_in, 32)  # ident arrives with the second load

            def tp(r):
                if r >= 2:
                    t.wait_ge(s_c, r - 1)  # ctp bank reader (mul of r-2) done
                t.wait_ge(s_cnt, r + 1)
                t.transpose(ctp[r % 2][:], cnt[r][:], ident_ap).then_inc(s_tp, 1)

            for r in range(RCH):
                tp(r)
                t.wait_ge(s_c, r + 1)  # ct[r] ready
                if r == 0:
                    t.wait_ge(s_w, 16)
                for j in range(NSL // 2):
                    u = r * (NSL // 2) + j
                    if u >= NPS:
                        unit_done_waits(t, u - NPS)
                    for h in range(2):
                        s = 2 * j + h
                        t.matmul(
                            pm[u % NPS][:, h * D : (h + 1) * D],
                            ct[r][:],
                            w_t[:, s * D : (s + 1) * D],
                            start=True,
                            stop=True,
                        ).then_inc(s_mm, 1)

        @block.scalar
        def _(a):
            for r in range(RCH):
                base = 8 * r
                a.wait_ge(s_mm, base + 2)
                u0_cols = slice(0, 512 if r == 0 else 1024)
                a.copy(stg[r][:, u0_cols], pm[(4 * r) % NPS][:, u0_cols]).then_inc(
                    s_cpa, 1
                )
                a.wait_ge(s_mm, base + 4)
                a.copy(stg[r][:, 1024:2048], pm[(4 * r + 1) % NPS][:]).then_inc(
                    s_cpa, 1
                )  # noqa: E501
                a.wait_ge(s_mm, base + 8)
                a.copy(
                    stg[r][:, 3072:3584], pm[(4 * r + 3) % NPS][:, 0:512]
                ).then_inc(s_cpa, 1)

        @block.gpsimd
        def _(gp):
            for r in range(RCH):
                base = 8 * r
                gp.wait_ge(s_mm, base + 6)
                gp.tensor_copy(stg[r][:, 2048:3072], pm[(4 * r + 2) % NPS][:]).then_inc(
                    s_cpp, 1
                )
                gp.wait_ge(s_mm, base + 8)
                gp.tensor_copy(
                    stg[r][:, 3584:4096], pm[(4 * r + 3) % NPS][:, 512:1024]
                ).then_inc(s_cpp, 1)

    return nc


def _prep_inputs(selection_score, expert_indices, all_weight):
    scores = np.asarray(selection_score, dtype=np.float32)
    idx = np.asarray(expert_indices)
    w = np.asarray(all_weight, dtype=np.float32).reshape(E, NF).astype(np.float16)
    iota = np.tile(np.arange(E, dtype=np.float16), (128, 1))
    ident = np.eye(128, dtype=np.float16)

    in_maps = []
    for core in range(N_CORES):
        rg, cg = divmod(core, CG)
        rsl = slice(rg * ROWS, (rg + 1) * ROWS)
        scoT = scores[rsl].T.astype(np.float16)  # [64, 1024]
        idxp = np.ascontiguousarray(
            idx[rsl]
            .astype(np.float32)
            .reshape(RCH, 128, TOPK)
            .transpose(1, 0, 2)
            .reshape(128, RCH * TOPK)
        )
        pk = np.zeros((128, PKW), dtype=np.float16)
        pk[:, PK_IOTA:PK_IDX] = iota
        pk[:, PK_IDX:PK_ID] = idxp.view(np.float16)
        pk[:, PK_ID:PK_SCO] = ident
        pk[:E, PK_SCO:] = scoT[:, :512]
        pk[E:, PK_SCO:] = scoT[:, 512:]
        wk = np.ascontiguousarray(w[:, cg * COLS : (cg + 1) * COLS])
        in_maps.append({"pk": pk, "wk": wk})
    return in_maps


def _run(selection_score, expert_indices, all_weight, trace=False):
    from concourse.bass_utils import run_bass_kernel_spmd

    in_maps = _prep_inputs(selection_score, expert_indices, all_weight)
    if "nc" not in _cache:
        _cache["nc"] = _build_program()
    nc = _cache["nc"]

    r = run_bass_kernel_spmd(nc, in_maps, list(range(N_CORES)), trace=trace)
    full = np.empty((BS, NF), dtype=np.float32)
    for core in range(N_CORES):
        rg, cg = divmod(core, CG)
        full[rg * ROWS : (rg + 1) * ROWS, cg * COLS : (cg + 1) * COLS] = r.results[
            core
        ]["out"]
    return full.reshape(BS, PL, D), r


def kernel(selection_score, expert_indices, all_weight) -> np.ndarray:
    full, _ = _run(selection_score, expert_indices, all_weight, trace=False)
    return full


# revision 18
# speedup vs baseline: 2.0508x; 1.0136x over previous
"""MoE routing mixture kernel for Trainium2 (8 NeuronCores, SPMD).

Math: out[b] = sum_k selection_score[b, idx[b,k]] * all_weight[idx[b,k]]
Rewritten as a dense matmul: out = C @ W_flat, where
  C[b,e]    = selection_score[b,e] * |{k : idx[b,k]==e}|      ([2048, 64])
  W_flat    = all_weight.reshape(64, 16384)

Sharding: 8 cores = 2 row-groups x 4 col-groups. Each core produces a
[1024, 4096] tile of the [2048, 16384] output. The big store is fp16
(the problem is DMA-roofline bound: fp32 out would be 16.8 MB/core,
fp16 is 8.4 MB; W slice per core is [64, 4096] fp16 = 0.5 MB).

Per-core pipeline (raw Bass):
  SP  : three loads in dependency order (mini: iota+idx bits; rest:
        ident+scoresT; W), then 32 output stores of [128, 1024] fp16.
  DVE : per 128-row chunk r: 8x tensor_scalar is_equal (fp16, 4x DVE
        mode), one strided tensor_reduce summing the 8 one-hot maps,
        ct[r] = ctp * scoresT (PSUM->SBUF move of C^T fused with the
        score multiply), then one PSUM cast copy for the previous
        chunk.
  PE  : per chunk: transpose cnt[r] -> ctp (fp16 PSUM); 8 fp16 matmuls
        [64,128]^T @ [64,512] -> fp32 PSUM (1 cycle/row), one per bank
        of a 7-deep PSUM ring.
  ACT : 4 of the 8 per-chunk PSUM->SBUF fp32->fp16 cast copies.
  Pool: 3 of the 8 cast copies.

Output assembled on host: fp16 tiles -> fp32 [2048, 32, 512].
"""

import sys
from contextlib import ExitStack

import numpy as np

sys.path.insert(0, "/opt/trn_rl_repo")

BS, E, TOPK, PL, D = 2048, 64, 8, 32, 512
NF = PL * D  # 16384 flattened prompt*dim
N_CORES = 8
RG, CG = 2, 4  # row groups x col groups
ROWS = BS // RG  # 1024 rows per core
COLS = NF // CG  # 4096 cols per core
RCH = ROWS // 128  # 8 row chunks
NSL = COLS // D  # 8 matmul slices of 512 cols per chunk
NPS = 7  # psum ring of [128, 512] banks

# packed small-input layout (fp16 cols), loaded as two DMAs:
#   mini [0 : 192)   = [64 iota | 128 idx-as-f32-bits]   (unblocks DVE eqs)
#   rest [192 : 832) = [128 ident | 512 scoT2]
PK_IOTA = 0
PK_IDX = E
PK_ID = E + 2 * RCH * TOPK
PK_SCO = PK_ID + 128
PKW = PK_SCO + 512
PK_MINI = PK_ID  # boundary between the two loads

# per-chunk copy-unit engine assignment (slice j of 8):
#   ACT  : j in {0, 2, 3, 5}
#   Pool : j in {1, 4, 6}
#   DVE  : j == 7 (copied in the next chunk's DVE iteration)
_A_SLICES = (0, 2, 3, 5)
_P_SLICES = (1, 4, 6)

_cache: dict = {}


def _unit_done_waits(u):
    """[(sem_name, value), ...] proving cast copy of unit u completed."""
    g, j = divmod(u, NSL)
    if j in _A_SLICES:
        return [("A", 4 * g + _A_SLICES.index(j) + 1)]
    if j in _P_SLICES:
        return [("P", 3 * g + _P_SLICES.index(j) + 1)]
    return [("D", g + 1)]


def _build_program():
    import concourse.bass as bass
    import concourse.mybir as mybir

    f16 = mybir.dt.float16
    f32 = mybir.dt.float32
    eq_op = mybir.AluOpType.is_equal
    nc = bass.Bass()

    pk_d = nc.declare_dram_parameter("pk", [128, PKW], f16, isOutput=False)
    w_d = nc.declare_dram_parameter("wk", [E, COLS], f16, isOutput=False)
    out_d = nc.declare_dram_parameter("out", [ROWS, COLS], f16, isOutput=True)

    ctx = ExitStack()
    with ctx:
        sb = lambda tag, shape, dt=f16: ctx.enter_context(  # noqa: E731
            nc.sbuf_tensor(tag, shape, dt)
        )
        pk_t = sb("pk_t", [128, PKW])
        w_t = sb("w_t", [E, COLS])
        eq = [sb(f"eq{r}", [128, TOPK * E]) for r in range(RCH)]
        cnt = [sb(f"cnt{r}", [128, E]) for r in range(RCH)]
        ct = [sb(f"ct{r}", [E, 128]) for r in range(RCH)]
        stg = [sb(f"stg{r}", [128, COLS]) for r in range(RCH)]

        iota_ap = pk_t[:, PK_IOTA:PK_IDX]
        ident_ap = pk_t[:, PK_ID:PK_SCO]

        def idx_scalar(r, k):
            c = PK_IDX + 2 * (r * TOPK + k)
            return pk_t[:, c : c + 2].bitcast(f32)

        def scoT_ap(r):
            # scoT2[p, c]: p<64 -> scores.T[p, c]; p>=64 -> scores.T[p-64, 512+c]
            pbase = (r // 4) * E
            cbase = PK_SCO + (r % 4) * 128
            return pk_t[pbase : pbase + E, cbase : cbase + 128]

        ctp = ctx.enter_context(nc.psum_tensor("ctp", [E, 128], f16))
        pm = [
            ctx.enter_context(nc.psum_tensor(f"pm{i}", [128, D], f32))
            for i in range(NPS)
        ]

        s_in = ctx.enter_context(nc.semaphore("s_in"))
        s_w = ctx.enter_context(nc.semaphore("s_w"))
        s_cnt = ctx.enter_context(nc.semaphore("s_cnt"))
        s_tp = ctx.enter_context(nc.semaphore("s_tp"))
        s_c = ctx.enter_context(nc.semaphore("s_c"))
        s_mm = ctx.enter_context(nc.semaphore("s_mm"))
        s_cp = {
            "A": ctx.enter_context(nc.semaphore("s_cpa")),
            "P": ctx.enter_context(nc.semaphore("s_cpp")),
            "D": ctx.enter_context(nc.semaphore("s_cpd")),
        }
        s_out = ctx.enter_context(nc.semaphore("s_out"))

        def unit_waits(engobj, u):
            for eng, n in _unit_done_waits(u):
                engobj.wait_ge(s_cp[eng], n)

        block = ctx.enter_context(nc.Block())

        @block.sync
        def _(sp):
            sp.dma_start(out=pk_t[:, :PK_MINI], in_=pk_d[:, :PK_MINI]).then_inc(
                s_in, 16
            )
            sp.dma_start(out=pk_t[:, PK_MINI:], in_=pk_d[:, PK_MINI:]).then_inc(
                s_in, 16
            )
            sp.dma_start(out=w_t[:], in_=w_d[:]).then_inc(s_w, 16)
            for st in range(4 * RCH):
                r, q = divmod(st, 4)
                for u in (8 * r + 2 * q, 8 * r + 2 * q + 1):
                    unit_waits(sp, u)
                rows = slice(r * 128, (r + 1) * 128)
                cols = slice(q * 1024, (q + 1) * 1024)
                sp.dma_start(out=out_d[rows, cols], in_=stg[r][:, cols]).then_inc(
                    s_out, 16
                )
            sp.wait_ge(s_out, 16 * 4 * RCH)

        @block.vector
        def _(v):
            def dve_copy(g):
                # cast copy of unit 8g+7 (all of chunk g's matmuls are done
                # one chunk later, so no stall here)
                v.wait_ge(s_mm, 8 * (g + 1))
                v.tensor_copy(
                    stg[g][:, 7 * D : 8 * D], pm[(8 * g + 7) % NPS][:]
                ).then_inc(s_cp["D"], 1)

            v.wait_ge(s_in, 16)  # mini load: iota + idx bits
            for r in range(RCH):
                for k in range(TOPK):
                    v.tensor_scalar(
                        eq[r][:, k * E : (k + 1) * E],
                        iota_ap,
                        idx_scalar(r, k),
                        None,
                        eq_op,
                    )
                v.drain()
                # counts are small integers (<= 8): exact in fp16
                with nc.allow_low_precision(reason="counts <= 8 are exact in fp16"):
                    v.tensor_reduce(
                        cnt[r][:],
                        eq[r][:].rearrange("p (k e) -> p e k", k=TOPK),
                        mybir.AxisListType.X,
                        mybir.AluOpType.add,
                    ).then_inc(s_cnt, 1)
                if r == 0:
                    v.wait_ge(s_in, 32)  # scoT arrives with the second load
                v.wait_ge(s_tp, r + 1)
                v.tensor_mul(ct[r][:], ctp[:], scoT_ap(r)).then_inc(s_c, 1)
                if r >= 1:
                    dve_copy(r - 1)
            dve_copy(RCH - 1)

        @block.tensor
        def _(t):
            t.wait_ge(s_in, 32)  # ident arrives with the second load
            for r in range(RCH):
                if r >= 1:
                    t.wait_ge(s_c, r)  # ctp reader (mul of r-1) done
                t.wait_ge(s_cnt, r + 1)
                t.transpose(ctp[:], cnt[r][:], ident_ap).then_inc(s_tp, 1)
                t.wait_ge(s_c, r + 1)  # ct[r] ready
                if r == 0:
                    t.wait_ge(s_w, 16)
                for j in range(NSL):
                    m = r * NSL + j
                    if m >= NPS:
                        unit_waits(t, m - NPS)
                    t.matmul(
                        pm[m % NPS][:],
                        ct[r][:],
                        w_t[:, j * D : (j + 1) * D],
                        start=True,
                        stop=True,
                    ).then_inc(s_mm, 1)

        @block.scalar
        def _(a):
            for r in range(RCH):
                for j in _A_SLICES:
                    m = r * NSL + j
                    a.wait_ge(s_mm, m + 1)
                    a.copy(stg[r][:, j * D : (j + 1) * D], pm[m % NPS][:]).then_inc(
                        s_cp["A"], 1
                    )

        @block.gpsimd
        def _(gp):
            for r in range(RCH):
                for j in _P_SLICES:
                    m = r * NSL + j
                    gp.wait_ge(s_mm, m + 1)
                    gp.tensor_copy(
                        stg[r][:, j * D : (j + 1) * D], pm[m % NPS][:]
                    ).then_inc(s_cp["P"], 1)

    return nc


def _prep_inputs(selection_score, expert_indices, all_weight):
    scores = np.asarray(selection_score, dtype=np.float32)
    idx = np.asarray(expert_indices)
    w = np.asarray(all_weight, dtype=np.float32).reshape(E, NF).astype(np.float16)
    iota = np.tile(np.arange(E, dtype=np.float16), (128, 1))
    ident = np.eye(128, dtype=np.float16)

    in_maps = []
    for core in range(N_CORES):
        rg, cg = divmod(core, CG)
        rsl = slice(rg * ROWS, (rg + 1) * ROWS)
        scoT = scores[rsl].T.astype(np.float16)  # [64, 1024]
        idxp = np.ascontiguousarray(
            idx[rsl]
            .astype(np.float32)
            .reshape(RCH, 128, TOPK)
            .transpose(1, 0, 2)
            .reshape(128, RCH * TOPK)
        )
        pk = np.zeros((128, PKW), dtype=np.float16)
        pk[:, PK_IOTA:PK_IDX] = iota
        pk[:, PK_IDX:PK_ID] = idxp.view(np.float16)
        pk[:, PK_ID:PK_SCO] = ident
        pk[:E, PK_SCO:] = scoT[:, :512]
        pk[E:, PK_SCO:] = scoT[:, 512:]
        wk = np.ascontiguousarray(w[:, cg * COLS : (cg + 1) * COLS])
        in_maps.append({"pk": pk, "wk": wk})
    return in_maps


def _run(selection_score, expert_indices, all_weight, trace=False):
    from concourse.bass_utils import run_bass_kernel_spmd

    in_maps = _prep_inputs(selection_score, expert_indices, all_weight)
    if "nc" not in _cache:
        _cache["nc"] = _build_program()
    nc = _cache["nc"]

    r = run_bass_kernel_spmd(nc, in_maps, list(range(N_CORES)), trace=trace)
    full = np.empty((BS, NF), dtype=np.float32)
    for core in range(N_CORES):
        rg, cg = divmod(core, CG)
        full[rg * ROWS : (rg + 1) * ROWS, cg * COLS : (cg + 1) * COLS] = r.results[
            core
        ]["out"]
    return full.reshape(BS, PL, D), r


def kernel(selection_score, expert_indices, all_weight) -> np.ndarray:
    full, _ = _run(selection_score, expert_indices, all_weight, trace=False)
    return full


# revision 20
# speedup vs baseline: 2.0777x; 1.0131x over previous
"""MoE routing mixture kernel for Trainium2 (8 NeuronCores, SPMD).

Math: out[b] = sum_k selection_score[b, idx[b,k]] * all_weight[idx[b,k]]
Rewritten as a dense matmul: out = C @ W_flat, where
  C[b,e]    = selection_score[b,e] * |{k : idx[b,k]==e}|      ([2048, 64])
  W_flat    = all_weight.reshape(64, 16384)

Sharding: 8 cores = 2 row-groups x 4 col-groups. Each core produces a
[1024, 4096] tile of the [2048, 16384] output. The big store is fp16
(the problem is DMA-roofline bound: fp32 out would be 16.8 MB/core,
fp16 is 8.4 MB; W slice per core is [64, 4096] fp16 = 0.5 MB).

Per-core pipeline (raw Bass):
  SP  : three loads in dependency order (mini: iota+idx bits; rest:
        ident+scoresT; W), then 32 output stores of [128, 1024] fp16.
  DVE : per 128-row chunk r: 8x tensor_scalar is_equal (fp16, 4x DVE
        mode), one strided tensor_reduce summing the 8 one-hot maps,
        ct[r] = ctp * scoresT (PSUM->SBUF move of C^T fused with the
        score multiply), then one PSUM cast copy for the previous
        chunk.
  PE  : per chunk: transpose cnt[r] -> ctp (fp16 PSUM); 8 fp16 matmuls
        [64,128]^T @ [64,512] -> fp32 PSUM (1 cycle/row), one per bank
        of a 7-deep PSUM ring.
  ACT : 4 of the 8 per-chunk PSUM->SBUF fp32->fp16 cast copies.
  Pool: 3 of the 8 cast copies.

Output assembled on host: fp16 tiles -> fp32 [2048, 32, 512].
"""

import sys
from contextlib import ExitStack

import numpy as np

sys.path.insert(0, "/opt/trn_rl_repo")

BS, E, TOPK, PL, D = 2048, 64, 8, 32, 512
NF = PL * D  # 16384 flattened prompt*dim
N_CORES = 8
RG, CG = 2, 4  # row groups x col groups
ROWS = BS // RG  # 1024 rows per core
COLS = NF // CG  # 4096 cols per core
RCH = ROWS // 128  # 8 row chunks
NSL = COLS // D  # 8 matmul slices of 512 cols per chunk
NPS = 7  # psum ring of [128, 512] banks

# packed small-input layout (fp16 cols), loaded as two DMAs:
#   mini [0 : 192)   = [64 iota | 128 idx-as-f32-bits]   (unblocks DVE eqs)
#   rest [192 : 832) = [128 ident | 512 scoT2]
PK_IOTA = 0
PK_IDX = E
PK_ID = E + 2 * RCH * TOPK
PK_SCO = PK_ID + 128
PKW = PK_SCO + 512
PK_MINI = PK_ID  # boundary between the two loads

# per-chunk copy-unit engine assignment (slice j of 8). Chosen so each
# [128,1024] store's two units share an engine with consecutive ranks
# (single sem wait per store) and each unit's copy completes before the
# 7-deep PSUM ring needs its bank back.
#   ACT  : j in {0, 1, 4, 5}
#   Pool : j in {2, 3, 6}
#   DVE  : j == 7 (copied in the next chunk's DVE iteration)
_A_SLICES = (0, 1, 4, 5)
_P_SLICES = (2, 3, 6)

_cache: dict = {}


def _unit_done_waits(u):
    """[(sem_name, value), ...] proving cast copy of unit u completed."""
    g, j = divmod(u, NSL)
    if j in _A_SLICES:
        return [("A", 4 * g + _A_SLICES.index(j) + 1)]
    if j in _P_SLICES:
        return [("P", 3 * g + _P_SLICES.index(j) + 1)]
    return [("D", g + 1)]


def _build_program():
    import concourse.bass as bass
    import concourse.mybir as mybir

    f16 = mybir.dt.float16
    f32 = mybir.dt.float32
    eq_op = mybir.AluOpType.is_equal
    nc = bass.Bass()

    pk_d = nc.declare_dram_parameter("pk", [128, PKW], f16, isOutput=False)
    w_d = nc.declare_dram_parameter("wk", [E, COLS], f16, isOutput=False)
    out_d = nc.declare_dram_parameter("out", [ROWS, COLS], f16, isOutput=True)

    ctx = ExitStack()
    with ctx:
        sb = lambda tag, shape, dt=f16: ctx.enter_context(  # noqa: E731
            nc.sbuf_tensor(tag, shape, dt)
        )
        pk_t = sb("pk_t", [128, PKW])
        w_t = sb("w_t", [E, COLS])
        eq = [sb(f"eq{r}", [128, TOPK * E]) for r in range(RCH)]
        cnt = [sb(f"cnt{r}", [128, E]) for r in range(RCH)]
        ct = [sb(f"ct{r}", [E, 128]) for r in range(RCH)]
        stg = [sb(f"stg{r}", [128, COLS]) for r in range(RCH)]

        iota_ap = pk_t[:, PK_IOTA:PK_IDX]
        ident_ap = pk_t[:, PK_ID:PK_SCO]

        def idx_scalar(r, k):
            c = PK_IDX + 2 * (r * TOPK + k)
            return pk_t[:, c : c + 2].bitcast(f32)

        def scoT_ap(r):
            # scoT2[p, c]: p<64 -> scores.T[p, c]; p>=64 -> scores.T[p-64, 512+c]
            pbase = (r // 4) * E
            cbase = PK_SCO + (r % 4) * 128
            return pk_t[pbase : pbase + E, cbase : cbase + 128]

        ctp = ctx.enter_context(nc.psum_tensor("ctp", [E, 128], f16))
        pm = [
            ctx.enter_context(nc.psum_tensor(f"pm{i}", [128, D], f32))
            for i in range(NPS)
        ]

        s_in = ctx.enter_context(nc.semaphore("s_in"))
        s_w = ctx.enter_context(nc.semaphore("s_w"))
        s_cnt = ctx.enter_context(nc.semaphore("s_cnt"))
        s_tp = ctx.enter_context(nc.semaphore("s_tp"))
        s_c = ctx.enter_context(nc.semaphore("s_c"))
        s_mm = ctx.enter_context(nc.semaphore("s_mm"))
        s_cp = {
            "A": ctx.enter_context(nc.semaphore("s_cpa")),
            "P": ctx.enter_context(nc.semaphore("s_cpp")),
            "D": ctx.enter_context(nc.semaphore("s_cpd")),
        }
        s_out = ctx.enter_context(nc.semaphore("s_out"))

        def unit_waits(engobj, u):
            for eng, n in _unit_done_waits(u):
                engobj.wait_ge(s_cp[eng], n)

        block = ctx.enter_context(nc.Block())

        @block.sync
        def _(sp):
            sp.dma_start(out=pk_t[:, :PK_MINI], in_=pk_d[:, :PK_MINI]).then_inc(
                s_in, 16
            )
            sp.dma_start(out=pk_t[:, PK_MINI:], in_=pk_d[:, PK_MINI:]).then_inc(
                s_in, 16
            )
            sp.dma_start(out=w_t[:], in_=w_d[:]).then_inc(s_w, 16)
            for st in range(4 * RCH):
                r, q = divmod(st, 4)
                # units (2q, 2q+1) share an engine for q<3; q=3 is Pool+DVE
                if q == 0:
                    sp.wait_ge(s_cp["A"], 4 * r + 2)
                elif q == 1:
                    sp.wait_ge(s_cp["P"], 3 * r + 2)
                elif q == 2:
                    sp.wait_ge(s_cp["A"], 4 * r + 4)
                else:
                    sp.wait_ge(s_cp["P"], 3 * r + 3)
                    sp.wait_ge(s_cp["D"], r + 1)
                rows = slice(r * 128, (r + 1) * 128)
                cols = slice(q * 1024, (q + 1) * 1024)
                sp.dma_start(out=out_d[rows, cols], in_=stg[r][:, cols]).then_inc(
                    s_out, 16
                )
            sp.wait_ge(s_out, 16 * 4 * RCH)

        @block.vector
        def _(v):
            def dve_copy(g):
                # cast copy of unit 8g+7 (all of chunk g's matmuls are done
                # one chunk later, so no stall here)
                v.wait_ge(s_mm, 8 * (g + 1))
                v.tensor_copy(
                    stg[g][:, 7 * D : 8 * D], pm[(8 * g + 7) % NPS][:]
                ).then_inc(s_cp["D"], 1)

            v.wait_ge(s_in, 16)  # mini load: iota + idx bits
            for r in range(RCH):
                for k in range(TOPK):
                    v.tensor_scalar(
                        eq[r][:, k * E : (k + 1) * E],
                        iota_ap,
                        idx_scalar(r, k),
                        None,
                        eq_op,
                    )
                v.drain()
                # counts are small integers (<= 8): exact in fp16
                with nc.allow_low_precision(reason="counts <= 8 are exact in fp16"):
                    v.tensor_reduce(
                        cnt[r][:],
                        eq[r][:].rearrange("p (k e) -> p e k", k=TOPK),
                        mybir.AxisListType.X,
                        mybir.AluOpType.add,
                    ).then_inc(s_cnt, 1)
                if r == 0:
                    v.wait_ge(s_in, 32)  # scoT arrives with the second load
                v.wait_ge(s_tp, r + 1)
                v.tensor_mul(ct[r][:], ctp[:], scoT_ap(r)).then_inc(s_c, 1)
                if r >= 1:
                    dve_copy(r - 1)
            dve_copy(RCH - 1)

        @block.tensor
        def _(t):
            t.wait_ge(s_in, 32)  # ident arrives with the second load
            for r in range(RCH):
                if r >= 1:
                    t.wait_ge(s_c, r)  # ctp reader (mul of r-1) done
                t.wait_ge(s_cnt, r + 1)
                t.transpose(ctp[:], cnt[r][:], ident_ap).then_inc(s_tp, 1)
                t.wait_ge(s_c, r + 1)  # ct[r] ready
                if r == 0:
                    t.wait_ge(s_w, 16)
                for j in range(NSL):
                    m = r * NSL + j
                    if m >= NPS:
                        unit_waits(t, m - NPS)
                    t.matmul(
                        pm[m % NPS][:],
                        ct[r][:],
                        w_t[:, j * D : (j + 1) * D],
                        start=True,
                        stop=True,
                    ).then_inc(s_mm, 1)

        @block.scalar
        def _(a):
            for r in range(RCH):
                for j in _A_SLICES:
                    m = r * NSL + j
                    a.wait_ge(s_mm, m + 1)
                    a.copy(stg[r][:, j * D : (j + 1) * D], pm[m % NPS][:]).then_inc(
                        s_cp["A"], 1
                    )

        @block.gpsimd
        def _(gp):
            for r in range(RCH):
                for j in _P_SLICES:
                    m = r * NSL + j
                    gp.wait_ge(s_mm, m + 1)
                    gp.tensor_copy(
                        stg[r][:, j * D : (j + 1) * D], pm[m % NPS][:]
                    ).then_inc(s_cp["P"], 1)

    return nc


def _prep_inputs(selection_score, expert_indices, all_weight):
    scores = np.asarray(selection_score, dtype=np.float32)
    idx = np.asarray(expert_indices)
    w = np.asarray(all_weight, dtype=np.float32).reshape(E, NF).astype(np.float16)
    iota = np.tile(np.arange(E, dtype=np.float16), (128, 1))
    ident = np.eye(128, dtype=np.float16)

    in_maps = []
    for core in range(N_CORES):
        rg, cg = divmod(core, CG)
        rsl = slice(rg * ROWS, (rg + 1) * ROWS)
        scoT = scores[rsl].T.astype(np.float16)  # [64, 1024]
        idxp = np.ascontiguousarray(
            idx[rsl]
            .astype(np.float32)
            .reshape(RCH, 128, TOPK)
            .transpose(1, 0, 2)
            .reshape(128, RCH * TOPK)
        )
        pk = np.zeros((128, PKW), dtype=np.float16)
        pk[:, PK_IOTA:PK_IDX] = iota
        pk[:, PK_IDX:PK_ID] = idxp.view(np.float16)
        pk[:, PK_ID:PK_SCO] = ident
        pk[:E, PK_SCO:] = scoT[:, :512]
        pk[E:, PK_SCO:] = scoT[:, 512:]
        wk = np.ascontiguousarray(w[:, cg * COLS : (cg + 1) * COLS])
        in_maps.append({"pk": pk, "wk": wk})
    return in_maps


def _run(selection_score, expert_indices, all_weight, trace=False):
    from concourse.bass_utils import run_bass_kernel_spmd

    in_maps = _prep_inputs(selection_score, expert_indices, all_weight)
    if "nc" not in _cache:
        _cache["nc"] = _build_program()
    nc = _cache["nc"]

    r = run_bass_kernel_spmd(nc, in_maps, list(range(N_CORES)), trace=trace)
    full = np.empty((BS, NF), dtype=np.float32)
    for core in range(N_CORES):
        rg, cg = divmod(core, CG)
        full[rg * ROWS : (rg + 1) * ROWS, cg * COLS : (cg + 1) * COLS] = r.results[
            core
        ]["out"]
    return full.reshape(BS, PL, D), r


def kernel(selection_score, expert_indices, all_weight) -> np.ndarray:
    full, _ = _run(selection_score, expert_indices, all_weight, trace=False)
    return full


# revision 29
# speedup vs baseline: 2.0821x; 1.0021x over previous
"""MoE routing mixture kernel for Trainium2 (8 NeuronCores, SPMD).

Math: out[b] = sum_k selection_score[b, idx[b,k]] * all_weight[idx[b,k]]
Rewritten as a dense matmul: out = C @ W_flat, where
  C[b,e]    = selection_score[b,e] * |{k : idx[b,k]==e}|      ([2048, 64])
  W_flat    = all_weight.reshape(64, 16384)

Sharding: 8 cores = 2 row-groups x 4 col-groups. Each core produces a
[1024, 4096] tile of the [2048, 16384] output. The big store is fp16
(the problem is DMA-roofline bound: fp32 out would be 16.8 MB/core,
fp16 is 8.4 MB; W slice per core is [64, 4096] fp16 = 0.5 MB).

Engine constraints that shape the design: DMA cannot touch PSUM, and
GPSIMD (Pool) cannot access PSUM either, so every matmul result must
pass through an ACT or DVE PSUM->SBUF copy — those copies are the
second-tightest resource after the DMA. Pool therefore handles the
SBUF-only one-hot work.

Per-core pipeline (raw Bass):
  SP  : three loads in dependency order (mini: iota+idx bits; rest:
        ident+scoresT; W), then 32 output stores of [128, 1024] fp16.
  Pool: per 128-row chunk r>=1: 8x tensor_scalar is_equal + add tree
        -> cnt[r] (all SBUF).
  DVE : chunk 0's eq/reduce (fill path), then per chunk: ct[r] =
        ctp * scoresT (PSUM->SBUF move of C^T fused with the score
        multiply) + 3 of the 8 PSUM cast-copy slices.
  PE  : per chunk: transpose cnt[r] -> ctp (fp16 PSUM); 8 fp16 matmuls
        [64,128]^T @ [64,512] -> fp32 PSUM, rotating over a 7-bank
        PSUM ring (bank = matmul_index % 7).
  ACT : 5 of the 8 cast-copy slices per chunk.

Output assembled on host: fp16 tiles -> fp32 [2048, 32, 512].
"""

import sys
from contextlib import ExitStack

import numpy as np

sys.path.insert(0, "/opt/trn_rl_repo")

BS, E, TOPK, PL, D = 2048, 64, 8, 32, 512
NF = PL * D  # 16384 flattened prompt*dim
N_CORES = 8
RG, CG = 2, 4  # row groups x col groups
ROWS = BS // RG  # 1024 rows per core
COLS = NF // CG  # 4096 cols per core
RCH = ROWS // 128  # 8 row chunks
NSL = COLS // D  # 8 matmul slices of 512 cols per chunk
NPS = 7  # psum ring banks

# packed small-input layout (fp16 cols), loaded as two DMAs:
#   mini [0 : 192)   = [64 iota | 128 idx-as-f32-bits]   (unblocks eqs)
#   rest [192 : 832) = [128 ident | 512 scoT2]
PK_IOTA = 0
PK_IDX = E
PK_ID = E + 2 * RCH * TOPK
PK_SCO = PK_ID + 128
PKW = PK_SCO + 512
PK_MINI = PK_ID  # boundary between the two loads


def _group_units(g):
    """Copy units (tuples of slice indices) for group g, split by engine.

    The PSUM bank of matmul m is m % 7, so bank adjacency inside a group
    rotates with g; pairs are chosen so both slices sit in adjacent banks
    (single contiguous copy). ACT gets 5 slices (2 pairs + slice 7), DVE
    3 (1 pair + 1 single). The DVE single goes first when its ring
    deadline is tight (low slice index).
    """
    G = g % 7
    if G == 2:
        pairs, singles = [(0, 1), (2, 3), (5, 6)], [4, 7]
    elif G == 4:
        pairs, singles = [(0, 1), (3, 4), (5, 6)], [2, 7]
    elif G == 6:
        pairs, singles = [(1, 2), (3, 4), (5, 6)], [0, 7]
    else:
        pairs, singles = [(0, 1), (2, 3), (4, 5)], [6, 7]
    act = [pairs[0], pairs[2], (singles[1],)]
    if singles[0] < pairs[1][0]:
        dve = [(singles[0],), pairs[1]]
    else:
        dve = [pairs[1], (singles[0],)]
    return act, dve


def _copy_tables():
    """slice index (0..63) -> (engine, count) + per-group emission lists."""
    table = {}
    counts = {"A": 0, "D": 0}
    act_em, dve_em = [], []
    for g in range(RCH):
        act, dve = _group_units(g)
        act_em.append(act)
        dve_em.append(dve)
        for unit in act:
            counts["A"] += 1
            for s in unit:
                table[NSL * g + s] = ("A", counts["A"])
        for unit in dve:
            counts["D"] += 1
            for s in unit:
                table[NSL * g + s] = ("D", counts["D"])
    return table, act_em, dve_em


_SLICE_SEM, _ACT_EM, _DVE_EM = _copy_tables()

_cache: dict = {}


def _done_waits(*slices):
    need = {}
    for s in slices:
        sem, n = _SLICE_SEM[s]
        need[sem] = max(need.get(sem, 0), n)
    return sorted(need.items())


def _build_program():
    import concourse.bass as bass
    import concourse.mybir as mybir

    f16 = mybir.dt.float16
    f32 = mybir.dt.float32
    eq_op = mybir.AluOpType.is_equal
    add_op = mybir.AluOpType.add
    nc = bass.Bass()

    pk_d = nc.declare_dram_parameter("pk", [128, PKW], f16, isOutput=False)
    w_d = nc.declare_dram_parameter("wk", [E, COLS], f16, isOutput=False)
    out_d = nc.declare_dram_parameter("out", [ROWS, COLS], f16, isOutput=True)

    ctx = ExitStack()
    with ctx:
        sb = lambda tag, shape, dt=f16: ctx.enter_context(  # noqa: E731
            nc.sbuf_tensor(tag, shape, dt)
        )
        pk_t = sb("pk_t", [128, PKW])
        w_t = sb("w_t", [E, COLS])
        eq = [sb(f"eq{r}", [128, TOPK * E]) for r in range(RCH)]
        tr1 = sb("tr1", [128, 4 * E])
        tr2 = sb("tr2", [128, 2 * E])
        cnt = [sb(f"cnt{r}", [128, E]) for r in range(RCH)]
        ct = [sb(f"ct{r}", [E, 128]) for r in range(RCH)]
        stg = [sb(f"stg{r}", [128, COLS]) for r in range(RCH)]

        iota_ap = pk_t[:, PK_IOTA:PK_IDX]
        ident_ap = pk_t[:, PK_ID:PK_SCO]

        def idx_scalar(r, k):
            c = PK_IDX + 2 * (r * TOPK + k)
            return pk_t[:, c : c + 2].bitcast(f32)

        def scoT_ap(r):
            # scoT2[p, c]: p<64 -> scores.T[p, c]; p>=64 -> scores.T[p-64, 512+c]
            pbase = (r // 4) * E
            cbase = PK_SCO + (r % 4) * 128
            return pk_t[pbase : pbase + E, cbase : cbase + 128]

        ctp = ctx.enter_context(nc.psum_tensor("ctp", [E, 128], f16))
        pmall = ctx.enter_context(nc.psum_tensor("pmall", [128, NPS * D], f32))

        def pm_ap(bank, nbanks=1):
            return pmall[:, bank * D : (bank + nbanks) * D]

        s_in = ctx.enter_context(nc.semaphore("s_in"))
        s_w = ctx.enter_context(nc.semaphore("s_w"))
        s_cnt0 = ctx.enter_context(nc.semaphore("s_cnt0"))
        s_cnt = ctx.enter_context(nc.semaphore("s_cnt"))
        s_tp = ctx.enter_context(nc.semaphore("s_tp"))
        s_c = ctx.enter_context(nc.semaphore("s_c"))
        s_mm = ctx.enter_context(nc.semaphore("s_mm"))
        s_cp = {
            "A": ctx.enter_context(nc.semaphore("s_cpa")),
            "D": ctx.enter_context(nc.semaphore("s_cpd")),
        }
        s_out = ctx.enter_context(nc.semaphore("s_out"))

        def emit_unit_copy(engobj, copy_fn, sem, g, unit):
            """PSUM->SBUF fp32->fp16 cast copy of `unit` (adjacent banks)."""
            j0, jn = unit[0], unit[-1]
            m_last = NSL * g + jn
            engobj.wait_ge(s_mm, m_last + 1)
            bank = (NSL * g + j0) % NPS
            copy_fn(
                stg[g][:, j0 * D : (jn + 1) * D], pm_ap(bank, len(unit))
            ).then_inc(sem, 1)

        block = ctx.enter_context(nc.Block())

        @block.sync
        def _(sp):
            sp.dma_start(out=pk_t[:, :PK_MINI], in_=pk_d[:, :PK_MINI]).then_inc(
                s_in, 16
            )
            sp.dma_start(out=pk_t[:, PK_MINI:], in_=pk_d[:, PK_MINI:]).then_inc(
                s_in, 16
            )
            sp.dma_start(out=w_t[:], in_=w_d[:]).then_inc(s_w, 16)
            for st in range(4 * RCH):
                r, q = divmod(st, 4)
                for eng, n in _done_waits(NSL * r + 2 * q, NSL * r + 2 * q + 1):
                    sp.wait_ge(s_cp[eng], n)
                rows = slice(r * 128, (r + 1) * 128)
                cols = slice(q * 1024, (q + 1) * 1024)
                sp.dma_start(out=out_d[rows, cols], in_=stg[r][:, cols]).then_inc(
                    s_out, 16
                )
            sp.wait_ge(s_out, 16 * 4 * RCH)

        @block.gpsimd
        def _(gp):
            # one-hot counts for chunks 1..7 (chunk 0 runs on DVE for a
            # faster pipeline fill); all operands SBUF-only.
            gp.wait_ge(s_in, 16)
            for r in range(1, RCH):
                for k in range(TOPK):
                    gp.tensor_scalar(
                        eq[r][:, k * E : (k + 1) * E],
                        iota_ap,
                        idx_scalar(r, k),
                        None,
                        eq_op,
                    )
                gp.drain()
                gp.tensor_tensor(
                    tr1[:], eq[r][:, : 4 * E], eq[r][:, 4 * E :], add_op
                )
                gp.drain()
                gp.tensor_tensor(tr2[:], tr1[:, : 2 * E], tr1[:, 2 * E :], add_op)
                gp.drain()
                gp.tensor_tensor(cnt[r][:], tr2[:, :E], tr2[:, E:], add_op).then_inc(
                    s_cnt, 1
                )

        @block.vector
        def _(v):
            v.wait_ge(s_in, 16)  # mini load: iota + idx bits
            # chunk 0 count path (DVE is idle during fill)
            for k in range(TOPK):
                v.tensor_scalar(
                    eq[0][:, k * E : (k + 1) * E],
                    iota_ap,
                    idx_scalar(0, k),
                    None,
                    eq_op,
                )
            v.drain()
            # counts are small integers (<= 8): exact in fp16
            with nc.allow_low_precision(reason="counts <= 8 are exact in fp16"):
                v.tensor_reduce(
                    cnt[0][:],
                    eq[0][:].rearrange("p (k e) -> p e k", k=TOPK),
                    mybir.AxisListType.X,
                    add_op,
                ).then_inc(s_cnt0, 1)
            v.wait_ge(s_in, 32)  # scoT arrives with the second load
            for r in range(RCH):
                v.wait_ge(s_tp, r + 1)
                v.tensor_mul(ct[r][:], ctp[:], scoT_ap(r)).then_inc(s_c, 1)
                if r >= 1:
                    for unit in _DVE_EM[r - 1]:
                        emit_unit_copy(v, v.tensor_copy, s_cp["D"], r - 1, unit)
            for unit in _DVE_EM[RCH - 1]:
                emit_unit_copy(v, v.tensor_copy, s_cp["D"], RCH - 1, unit)

        @block.tensor
        def _(t):
            def tp(r):
                # ctp's previous reader (mul of r-1) is already done when
                # this runs: G(r-1) started, which required s_c >= r
                if r == 0:
                    t.wait_ge(s_cnt0, 1)
                else:
                    t.wait_ge(s_cnt, r)
                t.transpose(ctp[:], cnt[r][:], ident_ap).then_inc(s_tp, 1)

            t.wait_ge(s_in, 32)  # ident arrives with the second load
            tp(0)
            for r in range(RCH):
                t.wait_ge(s_c, r + 1)  # ct[r] ready
                if r == 0:
                    t.wait_ge(s_w, 16)
                for j in range(NSL):
                    if j == 2 and r + 1 < RCH:
                        # hoist next chunk's transpose between matmuls
                        tp(r + 1)
                    m = r * NSL + j
                    if m >= NPS:
                        # bank m%7 was last written by matmul m-7
                        for eng, n in _done_waits(m - NPS):
                            t.wait_ge(s_cp[eng], n)
                    t.matmul(
                        pm_ap(m % NPS),
                        ct[r][:],
                        w_t[:, j * D : (j + 1) * D],
                        start=True,
                        stop=True,
                    ).then_inc(s_mm, 1)

        @block.scalar
        def _(a):
            for r in range(RCH):
                for unit in _ACT_EM[r]:
                    emit_unit_copy(a, a.copy, s_cp["A"], r, unit)

    return nc


def _prep_inputs(selection_score, expert_indices, all_weight):
    scores = np.asarray(selection_score, dtype=np.float32)
    idx = np.asarray(expert_indices)
    w = np.asarray(all_weight, dtype=np.float32).reshape(E, NF).astype(np.float16)
    iota = np.tile(np.arange(E, dtype=np.float16), (128, 1))
    ident = np.eye(128, dtype=np.float16)

    in_maps = []
    for core in range(N_CORES):
        rg, cg = divmod(core, CG)
        rsl = slice(rg * ROWS, (rg + 1) * ROWS)
        scoT = scores[rsl].T.astype(np.float16)  # [64, 1024]
        idxp = np.ascontiguousarray(
            idx[rsl]
            .astype(np.float32)
            .reshape(RCH, 128, TOPK)
            .transpose(1, 0, 2)
            .reshape(128, RCH * TOPK)
        )
        pk = np.zeros((128, PKW), dtype=np.float16)
        pk[:, PK_IOTA:PK_IDX] = iota
        pk[:, PK_IDX:PK_ID] = idxp.view(np.float16)
        pk[:, PK_ID:PK_SCO] = ident
        pk[:E, PK_SCO:] = scoT[:, :512]
        pk[E:, PK_SCO:] = scoT[:, 512:]
        wk = np.ascontiguousarray(w[:, cg * COLS : (cg + 1) * COLS])
        in_maps.append({"pk": pk, "wk": wk})
    return in_maps


def _run(selection_score, expert_indices, all_weight, trace=False):
    from concourse.bass_utils import run_bass_kernel_spmd

    in_maps = _prep_inputs(selection_score, expert_indices, all_weight)
    if "nc" not in _cache:
        _cache["nc"] = _build_program()
    nc = _cache["nc"]

    r = run_bass_kernel_spmd(nc, in_maps, list(range(N_CORES)), trace=trace)
    full = np.empty((BS, NF), dtype=np.float32)
    for core in range(N_CORES):
        rg, cg = divmod(core, CG)
        full[rg * ROWS : (rg + 1) * ROWS, cg * COLS : (cg + 1) * COLS] = r.results[
            core
        ]["out"]
    return full.reshape(BS, PL, D), r


def kernel(selection_score, expert_indices, all_weight) -> np.ndarray:
    full, _ = _run(selection_score, expert_indices, all_weight, trace=False)
    return full


# revision 34
# speedup vs baseline: 2.0829x; 1.0004x over previous
"""MoE routing mixture kernel for Trainium2 (8 NeuronCores, SPMD).

Math: out[b] = sum_k selection_score[b, idx[b,k]] * all_weight[idx[b,k]]
Rewritten as a dense matmul: out = C @ W_flat, where
  C[b,e]    = selection_score[b,e] * |{k : idx[b,k]==e}|      ([2048, 64])
  W_flat    = all_weight.reshape(64, 16384)

Sharding: 8 cores = 2 row-groups x 4 col-groups. Each core produces a
[1024, 4096] tile of the [2048, 16384] output. The big store is fp16
(the problem is DMA-roofline bound: fp32 out would be 16.8 MB/core,
fp16 is 8.4 MB; W slice per core is [64, 4096] fp16 = 0.5 MB).

Engine constraints that shape the design: DMA cannot touch PSUM, and
GPSIMD (Pool) cannot access PSUM either, so every matmul result must
pass through an ACT or DVE PSUM->SBUF copy — those copies are the
second-tightest resource after the DMA. Pool therefore handles the
SBUF-only one-hot work.

Per-core pipeline (raw Bass):
  SP  : three loads in dependency order (mini: iota+idx bits; rest:
        ident+scoresT; W), then 32 output stores of [128, 1024] fp16.
  Pool: per 128-row chunk r>=1: 8x tensor_scalar is_equal + add tree
        -> cnt[r] (all SBUF).
  DVE : chunk 0's eq/reduce (fill path), then per chunk: ct[r] =
        ctp * scoresT (PSUM->SBUF move of C^T fused with the score
        multiply) + 3 of the 8 PSUM cast-copy slices.
  PE  : per chunk: transpose cnt[r] -> ctp (fp16 PSUM); 8 fp16 matmuls
        [64,128]^T @ [64,512] -> fp32 PSUM, rotating over a 7-bank
        PSUM ring (bank = matmul_index % 7).
  ACT : 5 of the 8 cast-copy slices per chunk.

Output assembled on host: fp16 tiles -> fp32 [2048, 32, 512].
"""

import sys
from contextlib import ExitStack

import numpy as np

sys.path.insert(0, "/opt/trn_rl_repo")

BS, E, TOPK, PL, D = 2048, 64, 8, 32, 512
NF = PL * D  # 16384 flattened prompt*dim
N_CORES = 8
RG, CG = 2, 4  # row groups x col groups
ROWS = BS // RG  # 1024 rows per core
COLS = NF // CG  # 4096 cols per core
RCH = ROWS // 128  # 8 row chunks
NSL = COLS // D  # 8 matmul slices of 512 cols per chunk
NPS = 7  # psum ring banks

# packed small-input layout (fp16 cols), loaded as two DMAs:
#   mini [0 : 192)   = [64 iota | 128 idx-as-f32-bits]   (unblocks eqs)
#   rest [192 : 832) = [128 ident | 512 scoT2]
PK_IOTA = 0
PK_IDX = E
PK_ID = E + 2 * RCH * TOPK
PK_SCO = PK_ID + 128
PKW = PK_SCO + 512
PK_MINI = PK_ID  # boundary between the two loads


def _group_units(g):
    """Copy units (tuples of slice indices) for group g, split by engine.

    The PSUM bank of matmul m is m % 7, so bank adjacency inside a group
    rotates with g; pairs are chosen so both slices sit in adjacent banks
    (single contiguous copy). ACT gets 5 slices (2 pairs + slice 7), DVE
    3 (1 pair + 1 single). The DVE single goes first when its ring
    deadline is tight (low slice index).
    """
    if g == 0:
        # fill path: all singles early; DVE does slice 0 inline right
        # after mul(0) while otherwise idle, so the first store (a [512]
        # store of slice 0) can issue as soon as possible
        return [(1,), (4, 5), (7,)], [(0,), (2, 3), (6,)]
    G = g % 7
    if G == 2:
        pairs, singles = [(0, 1), (2, 3), (5, 6)], [4, 7]
    elif G == 4:
        pairs, singles = [(0, 1), (3, 4), (5, 6)], [2, 7]
    elif G == 6:
        pairs, singles = [(1, 2), (3, 4), (5, 6)], [0, 7]
    else:
        pairs, singles = [(0, 1), (2, 3), (4, 5)], [6, 7]
    act = [pairs[0], pairs[2], (singles[1],)]
    if singles[0] < pairs[1][0]:
        dve = [(singles[0],), pairs[1]]
    else:
        dve = [pairs[1], (singles[0],)]
    return act, dve


def _copy_tables():
    """slice index (0..63) -> (engine, count) + per-group emission lists."""
    table = {}
    counts = {"A": 0, "D": 0}
    act_em, dve_em = [], []
    for g in range(RCH):
        act, dve = _group_units(g)
        act_em.append(act)
        dve_em.append(dve)
        for unit in act:
            counts["A"] += 1
            for s in unit:
                table[NSL * g + s] = ("A", counts["A"])
        for unit in dve:
            counts["D"] += 1
            for s in unit:
                table[NSL * g + s] = ("D", counts["D"])
    return table, act_em, dve_em


_SLICE_SEM, _ACT_EM, _DVE_EM = _copy_tables()

_cache: dict = {}


def _done_waits(*slices):
    need = {}
    for s in slices:
        sem, n = _SLICE_SEM[s]
        need[sem] = max(need.get(sem, 0), n)
    return sorted(need.items())


def _build_program():
    import concourse.bass as bass
    import concourse.mybir as mybir

    f16 = mybir.dt.float16
    f32 = mybir.dt.float32
    eq_op = mybir.AluOpType.is_equal
    add_op = mybir.AluOpType.add
    nc = bass.Bass()

    pk_d = nc.declare_dram_parameter("pk", [128, PKW], f16, isOutput=False)
    w_d = nc.declare_dram_parameter("wk", [E, COLS], f16, isOutput=False)
    out_d = nc.declare_dram_parameter("out", [ROWS, COLS], f16, isOutput=True)

    ctx = ExitStack()
    with ctx:
        sb = lambda tag, shape, dt=f16: ctx.enter_context(  # noqa: E731
            nc.sbuf_tensor(tag, shape, dt)
        )
        pk_t = sb("pk_t", [128, PKW])
        w_t = sb("w_t", [E, COLS])
        eq = [sb(f"eq{r}", [128, TOPK * E]) for r in range(RCH)]
        tr1 = sb("tr1", [128, 4 * E])
        tr2 = sb("tr2", [128, 2 * E])
        cnt = [sb(f"cnt{r}", [128, E]) for r in range(RCH)]
        ct = [sb(f"ct{r}", [E, 128]) for r in range(RCH)]
        stg = [sb(f"stg{r}", [128, COLS]) for r in range(RCH)]

        iota_ap = pk_t[:, PK_IOTA:PK_IDX]
        ident_ap = pk_t[:, PK_ID:PK_SCO]

        def idx_scalar(r, k):
            c = PK_IDX + 2 * (r * TOPK + k)
            return pk_t[:, c : c + 2].bitcast(f32)

        def scoT_ap(r):
            # scoT2[p, c]: p<64 -> scores.T[p, c]; p>=64 -> scores.T[p-64, 512+c]
            pbase = (r // 4) * E
            cbase = PK_SCO + (r % 4) * 128
            return pk_t[pbase : pbase + E, cbase : cbase + 128]

        ctp = ctx.enter_context(nc.psum_tensor("ctp", [E, 128], f16))
        pmall = ctx.enter_context(nc.psum_tensor("pmall", [128, NPS * D], f32))

        def pm_ap(bank, nbanks=1):
            return pmall[:, bank * D : (bank + nbanks) * D]

        s_in = ctx.enter_context(nc.semaphore("s_in"))
        s_w = ctx.enter_context(nc.semaphore("s_w"))
        s_cnt0 = ctx.enter_context(nc.semaphore("s_cnt0"))
        s_cnt = ctx.enter_context(nc.semaphore("s_cnt"))
        s_tp = ctx.enter_context(nc.semaphore("s_tp"))
        s_c = ctx.enter_context(nc.semaphore("s_c"))
        s_mm = ctx.enter_context(nc.semaphore("s_mm"))
        s_cp = {
            "A": ctx.enter_context(nc.semaphore("s_cpa")),
            "D": ctx.enter_context(nc.semaphore("s_cpd")),
        }
        s_out = ctx.enter_context(nc.semaphore("s_out"))

        def emit_unit_copy(engobj, copy_fn, sem, g, unit):
            """PSUM->SBUF fp32->fp16 cast copy of `unit` (adjacent banks)."""
            j0, jn = unit[0], unit[-1]
            m_last = NSL * g + jn
            engobj.wait_ge(s_mm, m_last + 1)
            bank = (NSL * g + j0) % NPS
            copy_fn(
                stg[g][:, j0 * D : (jn + 1) * D], pm_ap(bank, len(unit))
            ).then_inc(sem, 1)

        block = ctx.enter_context(nc.Block())

        @block.sync
        def _(sp):
            sp.dma_start(out=pk_t[:, :PK_MINI], in_=pk_d[:, :PK_MINI]).then_inc(
                s_in, 16
            )
            # W in halves bracketing the second pk load: chunk-0 matmuls
            # j<4 only need the first half, which lands before ct[0]
            sp.dma_start(out=w_t[:, : COLS // 2], in_=w_d[:, : COLS // 2]).then_inc(
                s_w, 16
            )
            sp.dma_start(out=pk_t[:, PK_MINI:], in_=pk_d[:, PK_MINI:]).then_inc(
                s_in, 16
            )
            sp.dma_start(out=w_t[:, COLS // 2 :], in_=w_d[:, COLS // 2 :]).then_inc(
                s_w, 16
            )
            # group 0 ships as 2x[512] + 3x[1024]; groups 1..7 as 4x[1024]
            stores = [(0, (0, 0)), (0, (1, 1)), (0, (2, 3)), (0, (4, 5)), (0, (6, 7))]
            stores += [
                (r, (2 * q, 2 * q + 1)) for r in range(1, RCH) for q in range(4)
            ]
            for r, (j0, j1) in stores:
                for eng, n in _done_waits(NSL * r + j0, NSL * r + j1):
                    sp.wait_ge(s_cp[eng], n)
                rows = slice(r * 128, (r + 1) * 128)
                cols = slice(j0 * D, (j1 + 1) * D)
                sp.dma_start(out=out_d[rows, cols], in_=stg[r][:, cols]).then_inc(
                    s_out, 16
                )
            sp.wait_ge(s_out, 16 * len(stores))

        @block.gpsimd
        def _(gp):
            # one-hot counts for chunks 1..7 (chunk 0 runs on DVE for a
            # faster pipeline fill); all operands SBUF-only.
            gp.wait_ge(s_in, 16)
            for r in range(1, RCH):
                for k in range(TOPK):
                    gp.tensor_scalar(
                        eq[r][:, k * E : (k + 1) * E],
                        iota_ap,
                        idx_scalar(r, k),
                        None,
                        eq_op,
                    )
                gp.drain()
                gp.tensor_tensor(
                    tr1[:], eq[r][:, : 4 * E], eq[r][:, 4 * E :], add_op
                )
                gp.drain()
                gp.tensor_tensor(tr2[:], tr1[:, : 2 * E], tr1[:, 2 * E :], add_op)
                gp.drain()
                gp.tensor_tensor(cnt[r][:], tr2[:, :E], tr2[:, E:], add_op).then_inc(
                    s_cnt, 1
                )

        @block.vector
        def _(v):
            v.wait_ge(s_in, 16)  # mini load: iota + idx bits
            # chunk 0 count path (DVE is idle during fill)
            for k in range(TOPK):
                v.tensor_scalar(
                    eq[0][:, k * E : (k + 1) * E],
                    iota_ap,
                    idx_scalar(0, k),
                    None,
                    eq_op,
                )
            v.drain()
            # counts are small integers (<= 8): exact in fp16
            with nc.allow_low_precision(reason="counts <= 8 are exact in fp16"):
                v.tensor_reduce(
                    cnt[0][:],
                    eq[0][:].rearrange("p (k e) -> p e k", k=TOPK),
                    mybir.AxisListType.X,
                    add_op,
                ).then_inc(s_cnt0, 1)
            v.wait_ge(s_in, 32)  # scoT arrives with the second pk load
            for r in range(RCH):
                v.wait_ge(s_tp, r + 1)
                v.tensor_mul(ct[r][:], ctp[:], scoT_ap(r)).then_inc(s_c, 1)
                if r == 0:
                    # slice 0 copy inline: DVE idle during fill
                    emit_unit_copy(v, v.tensor_copy, s_cp["D"], 0, _DVE_EM[0][0])
                else:
                    units = _DVE_EM[r - 1]
                    for unit in units[1:] if r == 1 else units:
                        emit_unit_copy(v, v.tensor_copy, s_cp["D"], r - 1, unit)
            for unit in _DVE_EM[RCH - 1]:
                emit_unit_copy(v, v.tensor_copy, s_cp["D"], RCH - 1, unit)

        @block.tensor
        def _(t):
            def tp(r):
                # ctp's previous reader (mul of r-1) is already done when
                # this runs: G(r-1) started, which required s_c >= r
                if r == 0:
                    t.wait_ge(s_cnt0, 1)
                else:
                    t.wait_ge(s_cnt, r)
                t.transpose(ctp[:], cnt[r][:], ident_ap).then_inc(s_tp, 1)

            t.wait_ge(s_in, 32)  # ident arrives with the second pk load
            tp(0)
            for r in range(RCH):
                t.wait_ge(s_c, r + 1)  # ct[r] ready
                if r == 0:
                    t.wait_ge(s_w, 16)  # first W half
                for j in range(NSL):
                    if r == 0 and j == 4:
                        t.wait_ge(s_w, 32)  # second W half
                    if j == 2 and r + 1 < RCH:
                        # hoist next chunk's transpose between matmuls
                        tp(r + 1)
                    m = r * NSL + j
                    if m >= NPS:
                        # bank m%7 was last written by matmul m-7
                        for eng, n in _done_waits(m - NPS):
                            t.wait_ge(s_cp[eng], n)
                    t.matmul(
                        pm_ap(m % NPS),
                        ct[r][:],
                        w_t[:, j * D : (j + 1) * D],
                        start=True,
                        stop=True,
                    ).then_inc(s_mm, 1)

        @block.scalar
        def _(a):
            for r in range(RCH):
                for unit in _ACT_EM[r]:
                    emit_unit_copy(a, a.copy, s_cp["A"], r, unit)

    return nc


def _prep_inputs(selection_score, expert_indices, all_weight):
    scores = np.asarray(selection_score, dtype=np.float32)
    idx = np.asarray(expert_indices)
    w = np.asarray(all_weight, dtype=np.float32).reshape(E, NF).astype(np.float16)
    iota = np.tile(np.arange(E, dtype=np.float16), (128, 1))
    ident = np.eye(128, dtype=np.float16)

    in_maps = []
    for core in range(N_CORES):
        rg, cg = divmod(core, CG)
        rsl = slice(rg * ROWS, (rg + 1) * ROWS)
        scoT = scores[rsl].T.astype(np.float16)  # [64, 1024]
        idxp = np.ascontiguousarray(
            idx[rsl]
            .astype(np.float32)
            .reshape(RCH, 128, TOPK)
            .transpose(1, 0, 2)
            .reshape(128, RCH * TOPK)
        )
        pk = np.zeros((128, PKW), dtype=np.float16)
        pk[:, PK_IOTA:PK_IDX] = iota
        pk[:, PK_IDX:PK_ID] = idxp.view(np.float16)
        pk[:, PK_ID:PK_SCO] = ident
        pk[:E, PK_SCO:] = scoT[:, :512]
        pk[E:, PK_SCO:] = scoT[:, 512:]
        wk = np.ascontiguousarray(w[:, cg * COLS : (cg + 1) * COLS])
        in_maps.append({"pk": pk, "wk": wk})
    return in_maps


def _run(selection_score, expert_indices, all_weight, trace=False):
    from concourse.bass_utils import run_bass_kernel_spmd

    in_maps = _prep_inputs(selection_score, expert_indices, all_weight)
    if "nc" not in _cache:
        _cache["nc"] = _build_program()
    nc = _cache["nc"]

    r = run_bass_kernel_spmd(nc, in_maps, list(range(N_CORES)), trace=trace)
    full = np.empty((BS, NF), dtype=np.float32)
    for core in range(N_CORES):
        rg, cg = divmod(core, CG)
        full[rg * ROWS : (rg + 1) * ROWS, cg * COLS : (cg + 1) * COLS] = r.results[
            core
        ]["out"]
    return full.reshape(BS, PL, D), r


def kernel(selection_score, expert_indices, all_weight) -> np.ndarray:
    full, _ = _run(selection_score, expert_indices, all_weight, trace=False)
    return full
